# revision 40
# baseline (speedup 1.0000x reference)
"""Trainium2 Bass kernel for nn_AttentionResidualBlock (B=16, C=256, H=W=32, heads=8).

Sharding: data-parallel over batch across 8 NeuronCores (2 images/core),
weights replicated.

Per core (per image):
  - conv3x3 as 9 shifted bf16 matmuls over a zero-padded [C, 34, 34] layout;
    BN scale folded into weights on host, BN shift + ReLU fused on DVE.
    Conv work paces as TensorE filler inside the attention slots.
  - attention head-by-head: scoresT[m,n] = k^T q via fp8e4 DoubleRow
    matmuls (k carries a zeroed second k-tile; both operands fp8) at half
    the bf16 streaming cost; exp on ScalarE (PSUM -> SBUF bf16); attn@v
    computed TRANSPOSED: out[n, d] = sum_m pt[m, n] v[m, d] with
    lhsT = pt blocks and a ones-augmented v (33 cols) so the softmax
    denominator lands on the same partition as its outputs. Normalize is
    a per-partition reciprocal + tensor_scalar multiply. The 8 psum
    accumulation groups per head run ncb-outer/mc-inner (sequential per
    bank) because start=True lazily re-arms the whole 2KB zero region.
  - attnT is transposed back to [c, n] with DMA XBAR transposes
    (SBUF->SBUF, no PSUM), then a dense out-projection; gate/out-bias/
    v-bias folded on host.
Conv/attn@v/proj matmuls bf16, scores fp8-DR, fp32 PSUM accumulation.
"""

import os
import numpy as np
import ml_dtypes
from contextlib import ExitStack

KDBG = bool(int(os.environ.get("KDBG", "0")))

import concourse.bass as bass
import concourse.bacc as bacc
import concourse.mybir as mybir
import concourse.tile as tile
from concourse.bass_utils import run_bass_kernel_spmd

F32 = mybir.dt.float32
BF16 = mybir.dt.bfloat16
FP8 = mybir.dt.float8e4
AF = mybir.ActivationFunctionType
ALU = mybir.AluOpType
DR = mybir.MatmulPerfMode.DoubleRow
EXP_SHIFT = -4.0  # exp(s-4): keeps fp8e4 probabilities in range
# fp8e4 Schraudolph constants for exp(s-4): bits = s*11.5416 + 9.4896, with
# uint8 saturation mapping s < -0.82 to p=0 (verified on HW; negligible mass)
E4_MULT, E4_ADD = 11.5416, 56.0 - 4 * 11.5416 - 0.344
# exp slots alternate DVE (uint8 Schraudolph) / Act (real exp -> fp8):
# idx % EXP_MOD < EXP_DVE go to DVE
EXP_MOD, EXP_DVE = 5, 2


def _dr0(ap: bass.AP) -> bass.AP:
    """Insert a stride-0 dim after the partition dim (DoubleRow k-tile reuse)."""
    return bass.AP(tensor=ap.tensor, offset=ap.offset,
                   ap=[list(ap.ap[0])] + [[0, 2]] + [list(d) for d in ap.ap[1:]])

C = 256
HEADS = 8
D = 32
B, H, W = 16, 32, 32
N = H * W          # 1024
HP = H + 2         # 34
EPS = 1e-5
N_CORES = 8
IMGS = B // N_CORES  # 2 images per core
CC = C // 128      # 2 channel chunks
MC = N // 128      # 8 spatial m-chunks
NCH = 8            # n-chunks for attn output
DAUG = D + 1       # 33 (v cols + ones col)

# packed bf16 weight layout (columns per partition)
W1_COLS = CC * 9 * CC * 128          # 4608
QKVO_COLS = CC * C                   # 512
PACK_COLS = 2 * W1_COLS + 4 * QKVO_COLS + 128  # w1 w2 q k v ow ident = 11392
O_IDENT = PACK_COLS - 128            # bf16 identity for PE-mode transpose
VEC_COLS = 3 * CC + 128 + 1          # shift1, shiftF, qbias, identity(f32), exp-shift


def build_nc() -> bass.Bass:
    nc = bacc.Bacc()

    x_d = nc.declare_dram_parameter("x_sh", [IMGS, CC, 128, HP * HP], BF16,
                                    isOutput=False)
    wp_d = nc.declare_dram_parameter("wpack", [128, PACK_COLS], BF16, isOutput=False)
    vec_d = nc.declare_dram_parameter("vecs", [128, VEC_COLS], F32, isOutput=False)
    out_d = nc.declare_dram_parameter("out_sh", [IMGS, CC, 128, N], F32, isOutput=True)
    if KDBG:
        dbgA = nc.declare_dram_parameter("dbg_A", [IMGS, CC, 128, N], BF16,
                                         isOutput=True)
        dbgT = nc.declare_dram_parameter("dbg_attnT", [IMGS, CC, 128, N], BF16,
                                         isOutput=True)
        dbgC = nc.declare_dram_parameter("dbg_c2x", [IMGS, CC, 128, N], F32,
                                         isOutput=True)

    o_w1, o_w2 = 0, W1_COLS
    o_q = 2 * W1_COLS
    o_k, o_v = o_q + QKVO_COLS, o_q + 2 * QKVO_COLS
    o_ow = o_q + 3 * QKVO_COLS

    with ExitStack() as ctx:
        tc = ctx.enter_context(tile.TileContext(nc))
        wpool = ctx.enter_context(tc.tile_pool(name="weights", bufs=1))
        xpool = ctx.enter_context(tc.tile_pool(name="acts", bufs=2))
        ptpool = ctx.enter_context(tc.tile_pool(name="pt", bufs=10))
        ps_sc = ctx.enter_context(tc.tile_pool(name="ps_sc", bufs=4, space="PSUM"))
        ps_at = ctx.enter_context(tc.tile_pool(name="ps_at", bufs=2, space="PSUM"))
        ps_cv = ctx.enter_context(tc.tile_pool(name="ps_cv", bufs=2, space="PSUM"))

        # ---- weights / vectors ----
        wpack = wpool.tile([128, PACK_COLS], BF16, tag="wpack")
        vecs = wpool.tile([128, VEC_COLS], F32, tag="vecs")

        def conv_w(base, ic, tap, oc):  # [128, 128] lhsT slice
            off = base + ((ic * 9 + tap) * CC + oc) * 128
            return wpack[:, off:off + 128]

        shift1 = lambda oc: vecs[:, oc:oc + 1]
        shiftF = lambda oc: vecs[:, CC + oc:CC + oc + 1]
        qbias = lambda oc: vecs[:, 2 * CC + oc:2 * CC + oc + 1]
        ident = vecs[:, 3 * CC:3 * CC + 128]
        expshift = vecs[:, 3 * CC + 128:3 * CC + 129]

        # ---- filler queue (PE work units paced into attention slots) ----
        queue = []

        def push(cycles, fn, front=False):
            queue_cycles[0] += cycles
            if front:
                queue.insert(0, (cycles, fn))
            else:
                queue.append((cycles, fn))

        queue_cycles = [0]   # running total of cycles in queue
        slots_left = [128]   # attention slots remaining in the whole program

        RESERVE = 7000  # PE cycles held back to fill the post-last-exp tail

        def pop_fill():
            # spread remaining queue work evenly over remaining slots, but
            # never burst past the Act period (starves the exp pacer) nor
            # drip so slowly that conv debt piles up past the last exp; keep
            # RESERVE cycles back so the tail chain (norm/transpose/proj)
            # overlaps PE work instead of idling it
            avail = max(0, queue_cycles[0] - RESERVE)
            budget = min(1600, max(1200, avail // max(1, slots_left[0])))
            budget = min(budget, avail)
            slots_left[0] -= 1
            done = 0
            while queue and done < budget:
                cyc, fn = queue.pop(0)
                queue_cycles[0] -= cyc
                fn()
                done += cyc

        def drain_queue():
            while queue:
                _, fn = queue.pop(0)
                fn()

        # ---- per-image tiles ----
        # x arrives pre-padded + pre-cast to bf16 from the host: contiguous
        # DMA, no border memsets, no on-device casts
        xtiles = {}

        def xload_dma(img):
            xpadb = xpool.tile([128, CC, HP, HP], BF16, tag="xpadb",
                               name=f"xpadb{img}")
            xtiles[img] = (xpadb, xpadb)

            def chunk(cc):
                nc.sync.dma_start(
                    out=xpadb[:, cc].rearrange("p r c -> p (r c)"),
                    in_=x_d[img, cc])
            return chunk

        def xflat(t, cc):  # unpadded [p, 32, 32] view
            return t[:, cc, 1:HP - 1, 1:HP - 1]

        # ---- qkv ----
        qkv_tiles = {}

        def qkv_alloc(img):
            # vaug: [p, mc-pair, pair-slot, head, 48] fp8 (cols 0:32 = v, 32 =
            # ones; 48-stride keeps the DR pair step 16B-aligned)
            d = {
                "q": xpool.tile([128, CC, N], FP8, tag="q", name=f"q{img}"),
                "k": xpool.tile([128, CC, 2, N], FP8, tag="k", name=f"k{img}"),
                "xnb": xpool.tile([128, CC, N], BF16, tag="xnb", name=f"xnb{img}"),
                "vaug": xpool.tile([128, MC // 2, 2, HEADS, 48], FP8, tag="vaug",
                                   name=f"vaug{img}"),
                "attnT": xpool.tile([128, CC, NCH * 128], BF16, tag="attnT",
                                    name=f"attnT{img}"),
                "A": xpool.tile([128, CC, N], BF16, tag="A", name=f"A{img}"),
            }
            # zero second k-tiles for the DoubleRow zero-pad trick (on DVE:
            # Pool's sequencer must stay clear for the startup x casts)
            nc.vector.memset(d["k"][:, :, 1, :], 0.0)
            qkv_tiles[img] = d
            return d

        def qk_chunk(img, oc, which):
            d = qkv_tiles[img]
            xpadb = xtiles[img][1]
            wb = o_q if which == "q" else o_k
            for nh in range(2):
                ps = ps_sc.tile([128, 512], F32, tag="sc",
                                name=f"ps{which}{img}_{oc}_{nh}")
                for ic in range(CC):
                    nc.tensor.matmul(
                        ps,
                        lhsT=wpack[:, wb + ic * C + oc * 128:
                                   wb + ic * C + (oc + 1) * 128],
                        rhs=xflat(xpadb, ic)[:, nh * 16:(nh + 1) * 16, :],
                        start=(ic == 0), stop=(ic == CC - 1))
                if which == "q":
                    nc.scalar.activation(d["q"][:, oc, nh * 512:(nh + 1) * 512],
                                         ps, AF.Identity, bias=qbias(oc))
                else:
                    nc.vector.tensor_copy(d["k"][:, oc, 0, nh * 512:(nh + 1) * 512],
                                          ps)

        def v_chunk(img, half):
            d = qkv_tiles[img]
            xpadb = xtiles[img][1]
            if half == 0:
                nc.gpsimd.memset(d["vaug"][:, :, :, :, D], 1.0)
                for cc in range(CC):
                    nc.gpsimd.tensor_copy(
                        d["xnb"][:, cc].rearrange("p (r c) -> p r c", r=H),
                        xflat(xpadb, cc))
            for pair in range(2):
                ps = ps_sc.tile([128, 512], F32, tag="sc",
                                name=f"psv{img}_{half}_{pair}")
                for sl in range(2):
                    mc = half * 4 + pair * 2 + sl
                    for ic in range(CC):
                        nc.tensor.matmul(
                            ps[:, sl * C:(sl + 1) * C],
                            lhsT=d["xnb"][:, ic, mc * 128:(mc + 1) * 128],
                            rhs=wpack[:, o_v + ic * C: o_v + (ic + 1) * C],
                            start=(ic == 0), stop=(ic == CC - 1))
                for sl in range(2):
                    mc = half * 4 + pair * 2 + sl
                    nc.vector.tensor_copy(
                        d["vaug"][:, mc // 2, mc % 2, :, 0:D],
                        ps[:, sl * C:(sl + 1) * C].rearrange("p (h e) -> p h e",
                                                             h=HEADS))

        # ---- conv chains (filler units) ----
        def push_conv_units(img, cname, w_base, oc, nh):
            state = {}
            mmlist = [(ic, tap) for ic in range(CC) for tap in range(9)]

            def consume(ps):
                xpad, xpadb = xtiles[img]
                if cname == "c1":
                    # on Act: Relu(ps + shift1) — Act has slack since 1/3 of
                    # the exp stream moved to DVE
                    nc.scalar.activation(
                        xflat(c1pads[img], oc)[:, nh * 16:(nh + 1) * 16, :],
                        ps.rearrange("p (r c) -> p r c", r=16),
                        AF.Relu, bias=shift1(oc))
                else:
                    nc.vector.scalar_tensor_tensor(
                        out=c2xs[img][:, oc, nh * 512:(nh + 1) * 512]
                            .rearrange("p (r c) -> p r c", r=16),
                        in0=ps.rearrange("p (r c) -> p r c", r=16),
                        scalar=shiftF(oc),
                        in1=xflat(xpadb, oc)[:, nh * 16:(nh + 1) * 16, :],
                        op0=ALU.add, op1=ALU.add)

            def mk(i0, i1):
                def fn():
                    if "ps" not in state:
                        state["ps"] = ps_cv.tile([128, 512], F32, tag="cv",
                                                 name=f"{cname}{img}_{oc}_{nh}")
                    ps = state["ps"]
                    src = xtiles[img][1] if cname == "c1" else c1pads[img]
                    for idx in range(i0, i1):
                        ic, tap = mmlist[idx]
                        ky, kx = divmod(tap, 3)
                        nc.tensor.matmul(
                            ps,
                            lhsT=conv_w(w_base, ic, tap, oc),
                            rhs=src[:, ic, ky + nh * 16:ky + nh * 16 + 16, kx:kx + W],
                            start=(idx == 0), stop=(idx == 17))
                    if i1 == 18:
                        consume(ps)
                return fn

            for i0 in range(0, 18, 3):
                push(3 * 512, mk(i0, min(i0 + 3, 18)))

        c1pads, c2xs = {}, {}

        def conv_alloc(img):
            c1pad = xpool.tile([128, CC, HP, HP], BF16, tag="c1pad", name=f"c1p{img}")
            for cc in range(CC):
                nc.gpsimd.memset(c1pad[:, cc, 0, :], 0.0)
                nc.gpsimd.memset(c1pad[:, cc, HP - 1, :], 0.0)
                nc.gpsimd.memset(c1pad[:, cc, 1:HP - 1, 0], 0.0)
                nc.gpsimd.memset(c1pad[:, cc, 1:HP - 1, HP - 1], 0.0)
            c1pads[img] = c1pad
            c2xs[img] = xpool.tile([128, CC, N], F32, tag="c2x", name=f"c2x{img}")

        def push_conv_all(img):
            for oc in range(CC):
                for nh in range(2):
                    push_conv_units(img, "c1", o_w1, oc, nh)
            for oc in range(CC):
                for nh in range(2):
                    push_conv_units(img, "c2", o_w2, oc, nh)

        # ---- attention ----
        def emit_head_norm(img, h, at):
            # one broadcast multiply per head: out[p,g,c] = at[p,g,c]*rcp[p,g]
            d = qkv_tiles[img]
            rcp = xpool.tile([128, NCH], F32, tag="rcp", name=f"rcp{img}_{h}")
            nc.vector.reciprocal(
                rcp, at.rearrange("p (g e) -> p g e", e=DAUG)[:, :, D])
            rcp_bc = bass.AP(tensor=rcp.tensor, offset=rcp.offset,
                             ap=[list(rcp.ap[0])] + [[1, NCH], [0, D]])
            cch, hh = h // 4, h % 4
            nc.vector.scalar_tensor_tensor(
                out=d["attnT"][:, cch].rearrange("p (g c) -> p g c", c=128)
                    [:, :, hh * D:(hh + 1) * D],
                in0=at.rearrange("p (g e) -> p g e", e=DAUG)[:, :, 0:D],
                scalar=0.0,
                in1=rcp_bc,
                op0=ALU.add, op1=ALU.mult)

        def push_transp(img, cc):
            # batched DMA XBAR transpose: ONE instruction flips all 8 128x128
            # blocks of a cc-half (SBUF->SBUF, no PSUM)
            d = qkv_tiles[img]

            # emitted IMMEDIATELY at the trigger point (not queued): proj fns
            # are front-pushed and would otherwise emit before this transpose,
            # reading A before it is written (no dep in the Tile trace)
            if img == IMGS - 1 and cc == 1:
                # tail: PE-mode transpose + Act copy -- PE and Act are idle
                # here, and this skips the ~2.4us DMA/DGE latency chain
                ps = ps_sc.tile([128, N], BF16, tag="sc", name="trtail")
                for b in range(NCH):
                    nc.tensor.transpose(
                        ps[:, b * 128:(b + 1) * 128],
                        d["attnT"][:, cc, b * 128:(b + 1) * 128],
                        wpack[:, O_IDENT:O_IDENT + 128])
                nc.scalar.activation(d["A"][:, cc], ps, AF.Identity)
            else:
                nc.sync.dma_start_transpose(
                    out=d["A"][:, cc].rearrange("p (a b) -> p a b", a=NCH),
                    in_=d["attnT"][:, cc])

        def push_proj(img):
            d = qkv_tiles[img]
            for oc in range(CC):
                for nh in range(2):
                    def fn(oc=oc, nh=nh):
                        pj = ps_cv.tile([128, 512], F32, tag="cv",
                                        name=f"pj{img}_{oc}_{nh}")
                        for cc in range(CC):
                            nc.tensor.matmul(
                                pj,
                                lhsT=wpack[:, o_ow + cc * C + oc * 128:
                                           o_ow + cc * C + oc * 128 + 128],
                                rhs=d["A"][:, cc, nh * 512:(nh + 1) * 512],
                                start=(cc == 0), stop=(cc == CC - 1))
                        cmb = xpool.tile([128, 512], F32, tag="cmb",
                                         name=f"cmb{img}_{oc}_{nh}")
                        nc.vector.scalar_tensor_tensor(
                            out=cmb, in0=pj, scalar=0.0,
                            in1=c2xs[img][:, oc, nh * 512:(nh + 1) * 512],
                            op0=ALU.add, op1=ALU.add)
                        osb = xpool.tile([128, 512], F32, tag="osb",
                                         name=f"osb{img}_{oc}_{nh}")
                        if img == IMGS - 1:
                            # tail: Act is idle after the last exp
                            nc.scalar.activation(osb, cmb, AF.Relu)
                        else:
                            nc.vector.tensor_scalar(osb, cmb, 0.0, None, ALU.max)
                        nc.sync.dma_start(
                            out=out_d[img, oc, :, nh * 512:(nh + 1) * 512], in_=osb)
                    # img0's proj flows as immediate filler; img1's drains at
                    # the tail after the conv reserve covers transpose latency
                    push(1024, fn, front=(img == 0))

        # attention sub-blocks: one (head, ncb) group per slot, lagged one
        # full head so all 8 pt tiles of the head exist. Groups are strictly
        # SEQUENTIAL within the psum bank (ncb outer, mc inner): TRN2's
        # start=True lazily re-arms the whole 2KB zero region, so interleaved
        # per-group starts would wipe other groups' partial accumulations.
        attn_subs = []
        at_tiles = {}

        def make_attn_subs(img, h, pts):
            d = qkv_tiles[img]

            def mk(ncb):
                def fn():
                    if ncb == 0:
                        at_tiles[(img, h)] = ps_at.tile(
                            [128, NCH * DAUG], F32, tag="at", name=f"at{img}_{h}")
                    at = at_tiles[(img, h)]
                    for pr in range(MC // 2):
                        # DoubleRow over a REAL mc-pair: pt pair tile is the
                        # lhsT k-pair, vaug pair-slots are the rhs
                        nc.tensor.matmul(
                            at[:, ncb * DAUG:(ncb + 1) * DAUG],
                            lhsT=pts[pr][:, :, ncb * 128:(ncb + 1) * 128],
                            rhs=d["vaug"][:, pr, :, h, 0:DAUG],
                            start=(pr == 0), stop=(pr == MC // 2 - 1),
                            perf_mode=DR, skip_group_check=True)
                    if ncb == NCH - 1:
                        emit_head_norm(img, h, at)
                        del at_tiles[(img, h)]
                        if h == 3:
                            push_transp(img, 0)
                        if h == 7:
                            push_transp(img, 1)
                            push_proj(img)
                return fn

            for ncb in range(NCH):
                attn_subs.append(mk(ncb))

        def head_loop(img):
            d = qkv_tiles[img]
            q_sb, k_sb = d["q"], d["k"]
            for h in range(HEADS):
                hp, cch = 32 * (h % 4), h // 4
                pts = []
                for mc in range(MC):
                    if mc % 2 == 0:
                        pts.append(ptpool.tile([128, 2, N], FP8, tag="pt",
                                               name=f"pt{img}_{h}_{mc // 2}"))
                    for nh in range(2):
                        # half-tile score psum (one bank) -> 4-deep rotation so
                        # the exp latency stays off the matmul critical path
                        sc = ps_sc.tile([128, 512], F32, tag="sc",
                                        name=f"sc{img}_{h}_{mc}_{nh}")
                        nc.tensor.matmul(
                            sc,
                            lhsT=k_sb[hp:hp + 32, cch, :, mc * 128:(mc + 1) * 128],
                            rhs=_dr0(q_sb[hp:hp + 32, cch,
                                          nh * 512:(nh + 1) * 512]),
                            start=True, stop=True, perf_mode=DR,
                            tile_position=(hp, 0))
                        slot = pts[-1][:, mc % 2, nh * 512:(nh + 1) * 512]
                        idx = ((img * HEADS + h) * MC + mc) * 2 + nh
                        if idx % EXP_MOD < EXP_DVE:
                            # fp8e4 Schraudolph on DVE: linear-bits in uint8;
                            # saturation maps s-4 < -10.4 to p=0 (tiny mass)
                            nc.vector.tensor_scalar(
                                slot.bitcast(mybir.dt.uint8), sc,
                                E4_MULT, E4_ADD, ALU.mult, ALU.add)
                        else:
                            nc.scalar.activation(slot, sc, AF.Exp, bias=expshift)
                    if attn_subs:
                        attn_subs.pop(0)()
                    pop_fill()
                make_attn_subs(img, h, pts)

        # ================= emission =================
        # DMA order matters: transfers serialize on the DMA engines, so the
        # small qkv weights + vecs go first (first consumers), then image 0,
        # then the two conv-weight halves, then image 1.
        chunk0 = xload_dma(0)
        nc.sync.dma_start(out=wpack[:, o_q:], in_=wp_d[:, o_q:])
        chunk0(0)
        chunk0(1)
        nc.sync.dma_start(out=vecs, in_=vec_d[:])
        nc.sync.dma_start(out=wpack[:, :W1_COLS], in_=wp_d[:, :W1_COLS])
        nc.sync.dma_start(out=wpack[:, W1_COLS:o_q], in_=wp_d[:, W1_COLS:o_q])
        chunk1 = xload_dma(1)
        chunk1(0)
        chunk1(1)

        qkv_alloc(0)
        qk_chunk(0, 0, "q")
        qk_chunk(0, 0, "k")
        push(0, lambda: conv_alloc(0))
        push(2048, lambda: v_chunk(0, 0), front=True)
        push(2048, lambda: v_chunk(0, 1))
        push(2048, lambda: qk_chunk(0, 1, "q"))
        push(2048, lambda: qk_chunk(0, 1, "k"))
        push_conv_all(0)
        # image 1 prep as filler inside image 0's slots
        push(0, lambda: (qkv_alloc(1), conv_alloc(1)) and None)
        push(2048, lambda: qk_chunk(1, 0, "q"))
        push(2048, lambda: qk_chunk(1, 0, "k"))
        push(2048, lambda: v_chunk(1, 0))
        push(2048, lambda: v_chunk(1, 1))
        push(2048, lambda: qk_chunk(1, 1, "q"))
        push(2048, lambda: qk_chunk(1, 1, "k"))

        head_loop(0)
        push_conv_all(1)
        head_loop(1)
        while attn_subs:
            attn_subs.pop(0)()
        drain_queue()
        if KDBG:
            for img in range(IMGS):
                d = qkv_tiles[img]
                for cc in range(CC):
                    nc.sync.dma_start(out=dbgA[img, cc], in_=d["A"][:, cc])
                    nc.sync.dma_start(out=dbgT[img, cc], in_=d["attnT"][:, cc])
                    nc.sync.dma_start(out=dbgC[img, cc], in_=c2xs[img][:, cc])

    nc.finalize()
    return nc


def _prep_inputs(inputs: dict) -> list[dict]:
    bf = ml_dtypes.bfloat16
    x = np.asarray(inputs["x"], dtype=np.float32)
    f32 = lambda k: np.asarray(inputs[k], dtype=np.float32)
    bn1_inv = f32("bn1_gamma") / np.sqrt(f32("bn1_var") + EPS)
    shift1 = f32("bn1_beta") - f32("bn1_mean") * bn1_inv + f32("conv1_b") * bn1_inv
    w1s = f32("conv1_w") * bn1_inv[:, None, None, None]
    bn2_inv = f32("bn2_gamma") / np.sqrt(f32("bn2_var") + EPS)
    shift2 = f32("bn2_beta") - f32("bn2_mean") * bn2_inv + f32("conv2_b") * bn2_inv
    w2s = f32("conv2_w") * bn2_inv[:, None, None, None]
    sg = 1.0 / (1.0 + np.exp(-float(np.asarray(inputs["gate"]))))
    ow = f32("out_w") * sg
    shiftF = shift2 + sg * f32("out_b") + sg * (f32("out_w") @ f32("v_b"))
    qws = f32("q_w") / np.sqrt(D)
    qbs = f32("q_b") / np.sqrt(D)

    def conv_pack(w):  # [O, I, 3, 3] -> [128, CC*9*CC*128]
        t = w.transpose(1, 2, 3, 0).reshape(CC, 128, 3, 3, CC, 128)
        return t.transpose(1, 0, 2, 3, 4, 5).reshape(128, W1_COLS)

    def pack_T(w):  # [O, C_in] -> [128, CC*C]
        return w.T.reshape(CC, 128, C).transpose(1, 0, 2).reshape(128, QKVO_COLS)

    wpack = np.concatenate(
        [conv_pack(w1s), conv_pack(w2s), pack_T(qws), pack_T(f32("k_w")),
         pack_T(f32("v_w")), pack_T(ow), np.eye(128, dtype=np.float32)],
        axis=1).astype(bf)
    assert wpack.shape == (128, PACK_COLS)

    vecs = np.concatenate(
        [np.stack([shift1.reshape(CC, 128), shiftF.reshape(CC, 128),
                   qbs.reshape(CC, 128)]).reshape(3 * CC, 128).T,
         np.eye(128, dtype=np.float32),
         np.full((128, 1), EXP_SHIFT, np.float32)], axis=1)
    assert vecs.shape == (128, VEC_COLS)
    shared = {"wpack": np.ascontiguousarray(wpack),
              "vecs": np.ascontiguousarray(vecs.astype(np.float32))}
    # pre-pad + pre-cast x to bf16: [B, CC, 128, HP, HP] with zero borders
    xp = np.zeros((B, CC, 128, HP, HP), dtype=bf)
    xp[:, :, :, 1:HP - 1, 1:HP - 1] = (
        x.reshape(B, CC, 128, H, W).astype(bf))
    xp = xp.reshape(B, CC, 128, HP * HP)
    in_maps = []
    for core in range(N_CORES):
        xs = xp[core * IMGS:(core + 1) * IMGS]
        in_maps.append({"x_sh": np.ascontiguousarray(xs), **shared})
    return in_maps


_NC_CACHE = {}


def _get_nc():
    if "nc" not in _NC_CACHE:
        _NC_CACHE["nc"] = build_nc()
    return _NC_CACHE["nc"]


def kernel(**inputs) -> np.ndarray:
    nc = _get_nc()
    in_maps = _prep_inputs(inputs)
    res = run_bass_kernel_spmd(nc, in_maps, core_ids=list(range(N_CORES)))
    outs = [res.results[i]["out_sh"].reshape(IMGS, C, H, W) for i in range(N_CORES)]
    return np.concatenate(outs, axis=0)



# revision 41
# speedup vs baseline: 1.0037x; 1.0037x over previous
"""Trainium2 Bass kernel for nn_AttentionResidualBlock (B=16, C=256, H=W=32, heads=8).

Sharding: data-parallel over batch across 8 NeuronCores (2 images/core),
weights replicated.

Per core (per image):
  - conv3x3 as 9 shifted bf16 matmuls over a zero-padded [C, 34, 34] layout;
    BN scale folded into weights on host, BN shift + ReLU fused on DVE.
    Conv work paces as TensorE filler inside the attention slots.
  - attention head-by-head: scoresT[m,n] = k^T q via fp8e4 DoubleRow
    matmuls (k carries a zeroed second k-tile; both operands fp8) at half
    the bf16 streaming cost; exp on ScalarE (PSUM -> SBUF bf16); attn@v
    computed TRANSPOSED: out[n, d] = sum_m pt[m, n] v[m, d] with
    lhsT = pt blocks and a ones-augmented v (33 cols) so the softmax
    denominator lands on the same partition as its outputs. Normalize is
    a per-partition reciprocal + tensor_scalar multiply. The 8 psum
    accumulation groups per head run ncb-outer/mc-inner (sequential per
    bank) because start=True lazily re-arms the whole 2KB zero region.
  - attnT is transposed back to [c, n] with DMA XBAR transposes
    (SBUF->SBUF, no PSUM), then a dense out-projection; gate/out-bias/
    v-bias folded on host.
Conv/attn@v/proj matmuls bf16, scores fp8-DR, fp32 PSUM accumulation.
"""

import os
import numpy as np
import ml_dtypes
from contextlib import ExitStack

KDBG = bool(int(os.environ.get("KDBG", "0")))

import concourse.bass as bass
import concourse.bacc as bacc
import concourse.mybir as mybir
import concourse.tile as tile
from concourse.bass_utils import run_bass_kernel_spmd

F32 = mybir.dt.float32
BF16 = mybir.dt.bfloat16
FP8 = mybir.dt.float8e4
AF = mybir.ActivationFunctionType
ALU = mybir.AluOpType
DR = mybir.MatmulPerfMode.DoubleRow
EXP_SHIFT = -4.0  # exp(s-4): keeps fp8e4 probabilities in range
# fp8e4 Schraudolph constants for exp(s-4): bits = s*11.5416 + 9.4896, with
# uint8 saturation mapping s < -0.82 to p=0 (verified on HW; negligible mass)
E4_MULT, E4_ADD = 11.5416, 56.0 - 4 * 11.5416 - 0.344
# exp slots alternate DVE (uint8 Schraudolph) / Act (real exp -> fp8):
# idx % EXP_MOD < EXP_DVE go to DVE
EXP_MOD, EXP_DVE = 5, 2


def _dr0(ap: bass.AP) -> bass.AP:
    """Insert a stride-0 dim after the partition dim (DoubleRow k-tile reuse)."""
    return bass.AP(tensor=ap.tensor, offset=ap.offset,
                   ap=[list(ap.ap[0])] + [[0, 2]] + [list(d) for d in ap.ap[1:]])

C = 256
HEADS = 8
D = 32
B, H, W = 16, 32, 32
N = H * W          # 1024
HP = H + 2         # 34
EPS = 1e-5
N_CORES = 8
IMGS = B // N_CORES  # 2 images per core
CC = C // 128      # 2 channel chunks
MC = N // 128      # 8 spatial m-chunks
NCH = 8            # n-chunks for attn output
DAUG = D + 1       # 33 (v cols + ones col)

# packed bf16 weight layout (columns per partition)
W1_COLS = CC * 9 * CC * 128          # 4608
QKVO_COLS = CC * C                   # 512
PACK_COLS = 2 * W1_COLS + 4 * QKVO_COLS + 128  # w1 w2 q k v ow ident = 11392
O_IDENT = PACK_COLS - 128            # bf16 identity for PE-mode transpose
VEC_COLS = 3 * CC + 128 + 1          # shift1, shiftF, qbias, identity(f32), exp-shift


def build_nc() -> bass.Bass:
    nc = bacc.Bacc()

    x_d = nc.declare_dram_parameter("x_sh", [IMGS, CC, 128, HP * HP], BF16,
                                    isOutput=False)
    wp_d = nc.declare_dram_parameter("wpack", [128, PACK_COLS], BF16, isOutput=False)
    vec_d = nc.declare_dram_parameter("vecs", [128, VEC_COLS], F32, isOutput=False)
    out_d = nc.declare_dram_parameter("out_sh", [IMGS, CC, 128, N], F32, isOutput=True)
    if KDBG:
        dbgA = nc.declare_dram_parameter("dbg_A", [IMGS, CC, 128, N], BF16,
                                         isOutput=True)
        dbgT = nc.declare_dram_parameter("dbg_attnT", [IMGS, CC, 128, N], BF16,
                                         isOutput=True)
        dbgC = nc.declare_dram_parameter("dbg_c2x", [IMGS, CC, 128, N], F32,
                                         isOutput=True)

    o_w1, o_w2 = 0, W1_COLS
    o_q = 2 * W1_COLS
    o_k, o_v = o_q + QKVO_COLS, o_q + 2 * QKVO_COLS
    o_ow = o_q + 3 * QKVO_COLS

    with ExitStack() as ctx:
        tc = ctx.enter_context(tile.TileContext(nc))
        wpool = ctx.enter_context(tc.tile_pool(name="weights", bufs=1))
        xpool = ctx.enter_context(tc.tile_pool(name="acts", bufs=2))
        ptpool = ctx.enter_context(tc.tile_pool(name="pt", bufs=10))
        ps_sc = ctx.enter_context(tc.tile_pool(name="ps_sc", bufs=4, space="PSUM"))
        ps_at = ctx.enter_context(tc.tile_pool(name="ps_at", bufs=2, space="PSUM"))
        ps_cv = ctx.enter_context(tc.tile_pool(name="ps_cv", bufs=2, space="PSUM"))

        # ---- weights / vectors ----
        wpack = wpool.tile([128, PACK_COLS], BF16, tag="wpack")
        vecs = wpool.tile([128, VEC_COLS], F32, tag="vecs")

        def conv_w(base, ic, tap, oc):  # [128, 128] lhsT slice
            off = base + ((ic * 9 + tap) * CC + oc) * 128
            return wpack[:, off:off + 128]

        shift1 = lambda oc: vecs[:, oc:oc + 1]
        shiftF = lambda oc: vecs[:, CC + oc:CC + oc + 1]
        qbias = lambda oc: vecs[:, 2 * CC + oc:2 * CC + oc + 1]
        ident = vecs[:, 3 * CC:3 * CC + 128]
        expshift = vecs[:, 3 * CC + 128:3 * CC + 129]

        # ---- filler queue (PE work units paced into attention slots) ----
        queue = []

        def push(cycles, fn, front=False):
            queue_cycles[0] += cycles
            if front:
                queue.insert(0, (cycles, fn))
            else:
                queue.append((cycles, fn))

        queue_cycles = [0]   # running total of cycles in queue
        slots_left = [128]   # attention slots remaining in the whole program

        RESERVE = 7000  # PE cycles held back to fill the post-last-exp tail

        def pop_fill():
            # spread remaining queue work evenly over remaining slots, but
            # never burst past the Act period (starves the exp pacer) nor
            # drip so slowly that conv debt piles up past the last exp; keep
            # RESERVE cycles back so the tail chain (norm/transpose/proj)
            # overlaps PE work instead of idling it
            avail = max(0, queue_cycles[0] - RESERVE)
            budget = min(1600, max(1200, avail // max(1, slots_left[0])))
            budget = min(budget, avail)
            slots_left[0] -= 1
            done = 0
            while queue and done < budget:
                cyc, fn = queue.pop(0)
                queue_cycles[0] -= cyc
                fn()
                done += cyc

        def drain_queue():
            while queue:
                _, fn = queue.pop(0)
                fn()

        # ---- per-image tiles ----
        # x arrives pre-padded + pre-cast to bf16 from the host: contiguous
        # DMA, no border memsets, no on-device casts
        xtiles = {}

        def xload_dma(img):
            xpadb = xpool.tile([128, CC, HP, HP], BF16, tag="xpadb",
                               name=f"xpadb{img}")
            xtiles[img] = (xpadb, xpadb)

            def chunk(cc):
                nc.sync.dma_start(
                    out=xpadb[:, cc].rearrange("p r c -> p (r c)"),
                    in_=x_d[img, cc])
            return chunk

        def xflat(t, cc):  # unpadded [p, 32, 32] view
            return t[:, cc, 1:HP - 1, 1:HP - 1]

        # ---- qkv ----
        qkv_tiles = {}

        def qkv_alloc(img):
            # vaug: [p, mc-pair, pair-slot, head, 48] fp8 (cols 0:32 = v, 32 =
            # ones; 48-stride keeps the DR pair step 16B-aligned)
            d = {
                "q": xpool.tile([128, CC, N], FP8, tag="q", name=f"q{img}"),
                "k": xpool.tile([128, CC, 2, N], FP8, tag="k", name=f"k{img}"),
                "xnb": xpool.tile([128, CC, N], BF16, tag="xnb", name=f"xnb{img}"),
                "vaug": xpool.tile([128, MC // 2, 2, HEADS, 48], FP8, tag="vaug",
                                   name=f"vaug{img}"),
                "attnT": xpool.tile([128, CC, NCH * 128], BF16, tag="attnT",
                                    name=f"attnT{img}"),
                "A": xpool.tile([128, CC, N], BF16, tag="A", name=f"A{img}"),
            }
            # zero second k-tiles for the DoubleRow zero-pad trick (on DVE:
            # Pool's sequencer must stay clear for the startup x casts)
            nc.vector.memset(d["k"][:, :, 1, :], 0.0)
            qkv_tiles[img] = d
            return d

        def qk_chunk(img, oc, which):
            d = qkv_tiles[img]
            xpadb = xtiles[img][1]
            wb = o_q if which == "q" else o_k
            for nh in range(2):
                ps = ps_sc.tile([128, 512], F32, tag="sc",
                                name=f"ps{which}{img}_{oc}_{nh}")
                for ic in range(CC):
                    nc.tensor.matmul(
                        ps,
                        lhsT=wpack[:, wb + ic * C + oc * 128:
                                   wb + ic * C + (oc + 1) * 128],
                        rhs=xflat(xpadb, ic)[:, nh * 16:(nh + 1) * 16, :],
                        start=(ic == 0), stop=(ic == CC - 1))
                if which == "q":
                    nc.scalar.activation(d["q"][:, oc, nh * 512:(nh + 1) * 512],
                                         ps, AF.Identity, bias=qbias(oc))
                else:
                    nc.vector.tensor_copy(d["k"][:, oc, 0, nh * 512:(nh + 1) * 512],
                                          ps)

        def v_chunk(img, half):
            d = qkv_tiles[img]
            xpadb = xtiles[img][1]
            if half == 0:
                nc.gpsimd.memset(d["vaug"][:, :, :, :, D], 1.0)
                for cc in range(CC):
                    nc.gpsimd.tensor_copy(
                        d["xnb"][:, cc].rearrange("p (r c) -> p r c", r=H),
                        xflat(xpadb, cc))
            for pair in range(2):
                ps = ps_sc.tile([128, 512], F32, tag="sc",
                                name=f"psv{img}_{half}_{pair}")
                for sl in range(2):
                    mc = half * 4 + pair * 2 + sl
                    for ic in range(CC):
                        nc.tensor.matmul(
                            ps[:, sl * C:(sl + 1) * C],
                            lhsT=d["xnb"][:, ic, mc * 128:(mc + 1) * 128],
                            rhs=wpack[:, o_v + ic * C: o_v + (ic + 1) * C],
                            start=(ic == 0), stop=(ic == CC - 1))
                for sl in range(2):
                    mc = half * 4 + pair * 2 + sl
                    nc.vector.tensor_copy(
                        d["vaug"][:, mc // 2, mc % 2, :, 0:D],
                        ps[:, sl * C:(sl + 1) * C].rearrange("p (h e) -> p h e",
                                                             h=HEADS))

        # ---- conv chains (filler units) ----
        def push_conv_units(img, cname, w_base, oc, nh):
            state = {}
            mmlist = [(ic, tap) for ic in range(CC) for tap in range(9)]

            def consume(ps):
                xpad, xpadb = xtiles[img]
                if cname == "c1":
                    # on Act: Relu(ps + shift1) — Act has slack since 1/3 of
                    # the exp stream moved to DVE
                    nc.scalar.activation(
                        xflat(c1pads[img], oc)[:, nh * 16:(nh + 1) * 16, :],
                        ps.rearrange("p (r c) -> p r c", r=16),
                        AF.Relu, bias=shift1(oc))
                else:
                    nc.vector.scalar_tensor_tensor(
                        out=c2xs[img][:, oc, nh * 512:(nh + 1) * 512]
                            .rearrange("p (r c) -> p r c", r=16),
                        in0=ps.rearrange("p (r c) -> p r c", r=16),
                        scalar=shiftF(oc),
                        in1=xflat(xpadb, oc)[:, nh * 16:(nh + 1) * 16, :],
                        op0=ALU.add, op1=ALU.add)

            def mk(i0, i1):
                def fn():
                    if "ps" not in state:
                        state["ps"] = ps_cv.tile([128, 512], F32, tag="cv",
                                                 name=f"{cname}{img}_{oc}_{nh}")
                    ps = state["ps"]
                    src = xtiles[img][1] if cname == "c1" else c1pads[img]
                    for idx in range(i0, i1):
                        ic, tap = mmlist[idx]
                        ky, kx = divmod(tap, 3)
                        nc.tensor.matmul(
                            ps,
                            lhsT=conv_w(w_base, ic, tap, oc),
                            rhs=src[:, ic, ky + nh * 16:ky + nh * 16 + 16, kx:kx + W],
                            start=(idx == 0), stop=(idx == 17))
                    if i1 == 18:
                        consume(ps)
                return fn

            for i0 in range(0, 18, 3):
                push(3 * 512, mk(i0, min(i0 + 3, 18)))

        c1pads, c2xs = {}, {}

        def conv_alloc(img):
            c1pad = xpool.tile([128, CC, HP, HP], BF16, tag="c1pad", name=f"c1p{img}")
            for cc in range(CC):
                nc.gpsimd.memset(c1pad[:, cc, 0, :], 0.0)
                nc.gpsimd.memset(c1pad[:, cc, HP - 1, :], 0.0)
                nc.gpsimd.memset(c1pad[:, cc, 1:HP - 1, 0], 0.0)
                nc.gpsimd.memset(c1pad[:, cc, 1:HP - 1, HP - 1], 0.0)
            c1pads[img] = c1pad
            c2xs[img] = xpool.tile([128, CC, N], F32, tag="c2x", name=f"c2x{img}")

        def push_conv_all(img):
            for oc in range(CC):
                for nh in range(2):
                    push_conv_units(img, "c1", o_w1, oc, nh)
            for oc in range(CC):
                for nh in range(2):
                    push_conv_units(img, "c2", o_w2, oc, nh)

        # ---- attention ----
        def emit_head_norm(img, h, at):
            # one broadcast multiply per head: out[p,g,c] = at[p,g,c]*rcp[p,g]
            d = qkv_tiles[img]
            rcp = xpool.tile([128, NCH], F32, tag="rcp", name=f"rcp{img}_{h}")
            nc.vector.reciprocal(
                rcp, at.rearrange("p (g e) -> p g e", e=DAUG)[:, :, D])
            rcp_bc = bass.AP(tensor=rcp.tensor, offset=rcp.offset,
                             ap=[list(rcp.ap[0])] + [[1, NCH], [0, D]])
            cch, hh = h // 4, h % 4
            nc.vector.scalar_tensor_tensor(
                out=d["attnT"][:, cch].rearrange("p (g c) -> p g c", c=128)
                    [:, :, hh * D:(hh + 1) * D],
                in0=at.rearrange("p (g e) -> p g e", e=DAUG)[:, :, 0:D],
                scalar=0.0,
                in1=rcp_bc,
                op0=ALU.add, op1=ALU.mult)

        def push_transp(img, cc):
            # batched DMA XBAR transpose: ONE instruction flips all 8 128x128
            # blocks of a cc-half (SBUF->SBUF, no PSUM)
            d = qkv_tiles[img]

            # emitted IMMEDIATELY at the trigger point (not queued): proj fns
            # are front-pushed and would otherwise emit before this transpose,
            # reading A before it is written (no dep in the Tile trace)
            if img == IMGS - 1 and cc == 1:
                # tail: PE-mode transpose + Act copy -- PE and Act are idle
                # here, and this skips the ~2.4us DMA/DGE latency chain
                ps = ps_sc.tile([128, N], BF16, tag="sc", name="trtail")
                for b in range(NCH):
                    nc.tensor.transpose(
                        ps[:, b * 128:(b + 1) * 128],
                        d["attnT"][:, cc, b * 128:(b + 1) * 128],
                        wpack[:, O_IDENT:O_IDENT + 128])
                nc.scalar.activation(d["A"][:, cc], ps, AF.Identity)
            else:
                nc.sync.dma_start_transpose(
                    out=d["A"][:, cc].rearrange("p (a b) -> p a b", a=NCH),
                    in_=d["attnT"][:, cc])

        def push_proj(img):
            d = qkv_tiles[img]
            for oc in range(CC):
                for nh in range(2):
                    def fn(oc=oc, nh=nh):
                        pj = ps_cv.tile([128, 512], F32, tag="cv",
                                        name=f"pj{img}_{oc}_{nh}")
                        for cc in range(CC):
                            nc.tensor.matmul(
                                pj,
                                lhsT=wpack[:, o_ow + cc * C + oc * 128:
                                           o_ow + cc * C + oc * 128 + 128],
                                rhs=d["A"][:, cc, nh * 512:(nh + 1) * 512],
                                start=(cc == 0), stop=(cc == CC - 1))
                        cmb = xpool.tile([128, 512], F32, tag="cmb",
                                         name=f"cmb{img}_{oc}_{nh}")
                        nc.vector.scalar_tensor_tensor(
                            out=cmb, in0=pj, scalar=0.0,
                            in1=c2xs[img][:, oc, nh * 512:(nh + 1) * 512],
                            op0=ALU.add, op1=ALU.add)
                        osb = xpool.tile([128, 512], F32, tag="osb",
                                         name=f"osb{img}_{oc}_{nh}")
                        if img == IMGS - 1:
                            # tail: Act is idle after the last exp
                            nc.scalar.activation(osb, cmb, AF.Relu)
                        else:
                            nc.gpsimd.tensor_scalar(osb, cmb, 0.0, None, ALU.max)
                        nc.sync.dma_start(
                            out=out_d[img, oc, :, nh * 512:(nh + 1) * 512], in_=osb)
                    # img0's proj flows as immediate filler; img1's drains at
                    # the tail after the conv reserve covers transpose latency
                    push(1024, fn, front=(img == 0))

        # attention sub-blocks: one (head, ncb) group per slot, lagged one
        # full head so all 8 pt tiles of the head exist. Groups are strictly
        # SEQUENTIAL within the psum bank (ncb outer, mc inner): TRN2's
        # start=True lazily re-arms the whole 2KB zero region, so interleaved
        # per-group starts would wipe other groups' partial accumulations.
        attn_subs = []
        at_tiles = {}

        def make_attn_subs(img, h, pts):
            d = qkv_tiles[img]

            def mk(ncb):
                def fn():
                    if ncb == 0:
                        at_tiles[(img, h)] = ps_at.tile(
                            [128, NCH * DAUG], F32, tag="at", name=f"at{img}_{h}")
                    at = at_tiles[(img, h)]
                    for pr in range(MC // 2):
                        # DoubleRow over a REAL mc-pair: pt pair tile is the
                        # lhsT k-pair, vaug pair-slots are the rhs
                        nc.tensor.matmul(
                            at[:, ncb * DAUG:(ncb + 1) * DAUG],
                            lhsT=pts[pr][:, :, ncb * 128:(ncb + 1) * 128],
                            rhs=d["vaug"][:, pr, :, h, 0:DAUG],
                            start=(pr == 0), stop=(pr == MC // 2 - 1),
                            perf_mode=DR, skip_group_check=True)
                    if ncb == NCH - 1:
                        emit_head_norm(img, h, at)
                        del at_tiles[(img, h)]
                        if h == 3:
                            push_transp(img, 0)
                        if h == 7:
                            push_transp(img, 1)
                            push_proj(img)
                return fn

            for ncb in range(NCH):
                attn_subs.append(mk(ncb))

        def head_loop(img):
            d = qkv_tiles[img]
            q_sb, k_sb = d["q"], d["k"]
            for h in range(HEADS):
                hp, cch = 32 * (h % 4), h // 4
                pts = []
                for mc in range(MC):
                    if mc % 2 == 0:
                        pts.append(ptpool.tile([128, 2, N], FP8, tag="pt",
                                               name=f"pt{img}_{h}_{mc // 2}"))
                    for nh in range(2):
                        # half-tile score psum (one bank) -> 4-deep rotation so
                        # the exp latency stays off the matmul critical path
                        sc = ps_sc.tile([128, 512], F32, tag="sc",
                                        name=f"sc{img}_{h}_{mc}_{nh}")
                        nc.tensor.matmul(
                            sc,
                            lhsT=k_sb[hp:hp + 32, cch, :, mc * 128:(mc + 1) * 128],
                            rhs=_dr0(q_sb[hp:hp + 32, cch,
                                          nh * 512:(nh + 1) * 512]),
                            start=True, stop=True, perf_mode=DR,
                            tile_position=(hp, 0))
                        slot = pts[-1][:, mc % 2, nh * 512:(nh + 1) * 512]
                        idx = ((img * HEADS + h) * MC + mc) * 2 + nh
                        if idx % EXP_MOD < EXP_DVE:
                            # fp8e4 Schraudolph on DVE: linear-bits in uint8;
                            # saturation maps s-4 < -10.4 to p=0 (tiny mass)
                            nc.vector.tensor_scalar(
                                slot.bitcast(mybir.dt.uint8), sc,
                                E4_MULT, E4_ADD, ALU.mult, ALU.add)
                        else:
                            nc.scalar.activation(slot, sc, AF.Exp, bias=expshift)
                    if attn_subs:
                        attn_subs.pop(0)()
                    pop_fill()
                make_attn_subs(img, h, pts)

        # ================= emission =================
        # DMA order matters: transfers serialize on the DMA engines, so the
        # small qkv weights + vecs go first (first consumers), then image 0,
        # then the two conv-weight halves, then image 1.
        chunk0 = xload_dma(0)
        nc.sync.dma_start(out=wpack[:, o_q:], in_=wp_d[:, o_q:])
        chunk0(0)
        chunk0(1)
        nc.sync.dma_start(out=vecs, in_=vec_d[:])
        nc.sync.dma_start(out=wpack[:, :W1_COLS], in_=wp_d[:, :W1_COLS])
        nc.sync.dma_start(out=wpack[:, W1_COLS:o_q], in_=wp_d[:, W1_COLS:o_q])
        chunk1 = xload_dma(1)
        chunk1(0)
        chunk1(1)

        qkv_alloc(0)
        qk_chunk(0, 0, "q")
        qk_chunk(0, 0, "k")
        push(0, lambda: conv_alloc(0))
        push(2048, lambda: v_chunk(0, 0), front=True)
        push(2048, lambda: v_chunk(0, 1))
        push(2048, lambda: qk_chunk(0, 1, "q"))
        push(2048, lambda: qk_chunk(0, 1, "k"))
        push_conv_all(0)
        # image 1 prep as filler inside image 0's slots
        push(0, lambda: (qkv_alloc(1), conv_alloc(1)) and None)
        push(2048, lambda: qk_chunk(1, 0, "q"))
        push(2048, lambda: qk_chunk(1, 0, "k"))
        push(2048, lambda: v_chunk(1, 0))
        push(2048, lambda: v_chunk(1, 1))
        push(2048, lambda: qk_chunk(1, 1, "q"))
        push(2048, lambda: qk_chunk(1, 1, "k"))

        head_loop(0)
        push_conv_all(1)
        head_loop(1)
        while attn_subs:
            attn_subs.pop(0)()
        drain_queue()
        if KDBG:
            for img in range(IMGS):
                d = qkv_tiles[img]
                for cc in range(CC):
                    nc.sync.dma_start(out=dbgA[img, cc], in_=d["A"][:, cc])
                    nc.sync.dma_start(out=dbgT[img, cc], in_=d["attnT"][:, cc])
                    nc.sync.dma_start(out=dbgC[img, cc], in_=c2xs[img][:, cc])

    nc.finalize()
    return nc


def _prep_inputs(inputs: dict) -> list[dict]:
    bf = ml_dtypes.bfloat16
    x = np.asarray(inputs["x"], dtype=np.float32)
    f32 = lambda k: np.asarray(inputs[k], dtype=np.float32)
    bn1_inv = f32("bn1_gamma") / np.sqrt(f32("bn1_var") + EPS)
    shift1 = f32("bn1_beta") - f32("bn1_mean") * bn1_inv + f32("conv1_b") * bn1_inv
    w1s = f32("conv1_w") * bn1_inv[:, None, None, None]
    bn2_inv = f32("bn2_gamma") / np.sqrt(f32("bn2_var") + EPS)
    shift2 = f32("bn2_beta") - f32("bn2_mean") * bn2_inv + f32("conv2_b") * bn2_inv
    w2s = f32("conv2_w") * bn2_inv[:, None, None, None]
    sg = 1.0 / (1.0 + np.exp(-float(np.asarray(inputs["gate"]))))
    ow = f32("out_w") * sg
    shiftF = shift2 + sg * f32("out_b") + sg * (f32("out_w") @ f32("v_b"))
    qws = f32("q_w") / np.sqrt(D)
    qbs = f32("q_b") / np.sqrt(D)

    def conv_pack(w):  # [O, I, 3, 3] -> [128, CC*9*CC*128]
        t = w.transpose(1, 2, 3, 0).reshape(CC, 128, 3, 3, CC, 128)
        return t.transpose(1, 0, 2, 3, 4, 5).reshape(128, W1_COLS)

    def pack_T(w):  # [O, C_in] -> [128, CC*C]
        return w.T.reshape(CC, 128, C).transpose(1, 0, 2).reshape(128, QKVO_COLS)

    wpack = np.concatenate(
        [conv_pack(w1s), conv_pack(w2s), pack_T(qws), pack_T(f32("k_w")),
         pack_T(f32("v_w")), pack_T(ow), np.eye(128, dtype=np.float32)],
        axis=1).astype(bf)
    assert wpack.shape == (128, PACK_COLS)

    vecs = np.concatenate(
        [np.stack([shift1.reshape(CC, 128), shiftF.reshape(CC, 128),
                   qbs.reshape(CC, 128)]).reshape(3 * CC, 128).T,
         np.eye(128, dtype=np.float32),
         np.full((128, 1), EXP_SHIFT, np.float32)], axis=1)
    assert vecs.shape == (128, VEC_COLS)
    shared = {"wpack": np.ascontiguousarray(wpack),
              "vecs": np.ascontiguousarray(vecs.astype(np.float32))}
    # pre-pad + pre-cast x to bf16: [B, CC, 128, HP, HP] with zero borders
    xp = np.zeros((B, CC, 128, HP, HP), dtype=bf)
    xp[:, :, :, 1:HP - 1, 1:HP - 1] = (
        x.reshape(B, CC, 128, H, W).astype(bf))
    xp = xp.reshape(B, CC, 128, HP * HP)
    in_maps = []
    for core in range(N_CORES):
        xs = xp[core * IMGS:(core + 1) * IMGS]
        in_maps.append({"x_sh": np.ascontiguousarray(xs), **shared})
    return in_maps


_NC_CACHE = {}


def _get_nc():
    if "nc" not in _NC_CACHE:
        _NC_CACHE["nc"] = build_nc()
    return _NC_CACHE["nc"]


def kernel(**inputs) -> np.ndarray:
    nc = _get_nc()
    in_maps = _prep_inputs(inputs)
    res = run_bass_kernel_spmd(nc, in_maps, core_ids=list(range(N_CORES)))
    outs = [res.results[i]["out_sh"].reshape(IMGS, C, H, W) for i in range(N_CORES)]
    return np.concatenate(outs, axis=0)



# revision 42
# speedup vs baseline: 1.0041x; 1.0003x over previous
"""Trainium2 Bass kernel for nn_AttentionResidualBlock (B=16, C=256, H=W=32, heads=8).

Sharding: data-parallel over batch across 8 NeuronCores (2 images/core),
weights replicated.

Per core (per image):
  - conv3x3 as 9 shifted bf16 matmuls over a zero-padded [C, 34, 34] layout;
    BN scale folded into weights on host, BN shift + ReLU fused on DVE.
    Conv work paces as TensorE filler inside the attention slots.
  - attention head-by-head: scoresT[m,n] = k^T q via fp8e4 DoubleRow
    matmuls (k carries a zeroed second k-tile; both operands fp8) at half
    the bf16 streaming cost; exp on ScalarE (PSUM -> SBUF bf16); attn@v
    computed TRANSPOSED: out[n, d] = sum_m pt[m, n] v[m, d] with
    lhsT = pt blocks and a ones-augmented v (33 cols) so the softmax
    denominator lands on the same partition as its outputs. Normalize is
    a per-partition reciprocal + tensor_scalar multiply. The 8 psum
    accumulation groups per head run ncb-outer/mc-inner (sequential per
    bank) because start=True lazily re-arms the whole 2KB zero region.
  - attnT is transposed back to [c, n] with DMA XBAR transposes
    (SBUF->SBUF, no PSUM), then a dense out-projection; gate/out-bias/
    v-bias folded on host.
Conv/attn@v/proj matmuls bf16, scores fp8-DR, fp32 PSUM accumulation.
"""

import os
import numpy as np
import ml_dtypes
from contextlib import ExitStack

KDBG = bool(int(os.environ.get("KDBG", "0")))

import concourse.bass as bass
import concourse.bacc as bacc
import concourse.mybir as mybir
import concourse.tile as tile
from concourse.bass_utils import run_bass_kernel_spmd

F32 = mybir.dt.float32
BF16 = mybir.dt.bfloat16
FP8 = mybir.dt.float8e4
AF = mybir.ActivationFunctionType
ALU = mybir.AluOpType
DR = mybir.MatmulPerfMode.DoubleRow
EXP_SHIFT = -4.0  # exp(s-4): keeps fp8e4 probabilities in range
# fp8e4 Schraudolph constants for exp(s-4): bits = s*11.5416 + 9.4896, with
# uint8 saturation mapping s < -0.82 to p=0 (verified on HW; negligible mass)
E4_MULT, E4_ADD = 11.5416, 56.0 - 4 * 11.5416 - 0.344
# exp slots alternate DVE (uint8 Schraudolph) / Act (real exp -> fp8):
# idx % EXP_MOD < EXP_DVE go to DVE
EXP_MOD, EXP_DVE = 5, 2


def _dr0(ap: bass.AP) -> bass.AP:
    """Insert a stride-0 dim after the partition dim (DoubleRow k-tile reuse)."""
    return bass.AP(tensor=ap.tensor, offset=ap.offset,
                   ap=[list(ap.ap[0])] + [[0, 2]] + [list(d) for d in ap.ap[1:]])

C = 256
HEADS = 8
D = 32
B, H, W = 16, 32, 32
N = H * W          # 1024
HP = H + 2         # 34
EPS = 1e-5
N_CORES = 8
IMGS = B // N_CORES  # 2 images per core
CC = C // 128      # 2 channel chunks
MC = N // 128      # 8 spatial m-chunks
NCH = 8            # n-chunks for attn output
DAUG = D + 1       # 33 (v cols + ones col)

# packed bf16 weight layout (columns per partition)
W1_COLS = CC * 9 * CC * 128          # 4608
QKVO_COLS = CC * C                   # 512
PACK_COLS = 2 * W1_COLS + 4 * QKVO_COLS + 128  # w1 w2 q k v ow ident = 11392
O_IDENT = PACK_COLS - 128            # bf16 identity for PE-mode transpose
VEC_COLS = 3 * CC + 128 + 1          # shift1, shiftF, qbias, identity(f32), exp-shift


def build_nc() -> bass.Bass:
    nc = bacc.Bacc()

    x_d = nc.declare_dram_parameter("x_sh", [IMGS, CC, 128, HP * HP], BF16,
                                    isOutput=False)
    wp_d = nc.declare_dram_parameter("wpack", [128, PACK_COLS], BF16, isOutput=False)
    vec_d = nc.declare_dram_parameter("vecs", [128, VEC_COLS], F32, isOutput=False)
    out_d = nc.declare_dram_parameter("out_sh", [IMGS, CC, 128, N], F32, isOutput=True)
    if KDBG:
        dbgA = nc.declare_dram_parameter("dbg_A", [IMGS, CC, 128, N], BF16,
                                         isOutput=True)
        dbgT = nc.declare_dram_parameter("dbg_attnT", [IMGS, CC, 128, N], BF16,
                                         isOutput=True)
        dbgC = nc.declare_dram_parameter("dbg_c2x", [IMGS, CC, 128, N], F32,
                                         isOutput=True)

    o_w1, o_w2 = 0, W1_COLS
    o_q = 2 * W1_COLS
    o_k, o_v = o_q + QKVO_COLS, o_q + 2 * QKVO_COLS
    o_ow = o_q + 3 * QKVO_COLS

    with ExitStack() as ctx:
        tc = ctx.enter_context(tile.TileContext(nc))
        wpool = ctx.enter_context(tc.tile_pool(name="weights", bufs=1))
        xpool = ctx.enter_context(tc.tile_pool(name="acts", bufs=2))
        ptpool = ctx.enter_context(tc.tile_pool(name="pt", bufs=10))
        ps_sc = ctx.enter_context(tc.tile_pool(name="ps_sc", bufs=4, space="PSUM"))
        ps_at = ctx.enter_context(tc.tile_pool(name="ps_at", bufs=2, space="PSUM"))
        ps_cv = ctx.enter_context(tc.tile_pool(name="ps_cv", bufs=2, space="PSUM"))

        # ---- weights / vectors ----
        wpack = wpool.tile([128, PACK_COLS], BF16, tag="wpack")
        vecs = wpool.tile([128, VEC_COLS], F32, tag="vecs")

        def conv_w(base, ic, tap, oc):  # [128, 128] lhsT slice
            off = base + ((ic * 9 + tap) * CC + oc) * 128
            return wpack[:, off:off + 128]

        shift1 = lambda oc: vecs[:, oc:oc + 1]
        shiftF = lambda oc: vecs[:, CC + oc:CC + oc + 1]
        qbias = lambda oc: vecs[:, 2 * CC + oc:2 * CC + oc + 1]
        ident = vecs[:, 3 * CC:3 * CC + 128]
        expshift = vecs[:, 3 * CC + 128:3 * CC + 129]

        # ---- filler queue (PE work units paced into attention slots) ----
        queue = []

        def push(cycles, fn, front=False):
            queue_cycles[0] += cycles
            if front:
                queue.insert(0, (cycles, fn))
            else:
                queue.append((cycles, fn))

        queue_cycles = [0]   # running total of cycles in queue
        slots_left = [128]   # attention slots remaining in the whole program

        RESERVE = 7000  # PE cycles held back to fill the post-last-exp tail

        def pop_fill():
            # spread remaining queue work evenly over remaining slots, but
            # never burst past the Act period (starves the exp pacer) nor
            # drip so slowly that conv debt piles up past the last exp; keep
            # RESERVE cycles back so the tail chain (norm/transpose/proj)
            # overlaps PE work instead of idling it
            avail = max(0, queue_cycles[0] - RESERVE)
            budget = min(1600, max(1200, avail // max(1, slots_left[0])))
            budget = min(budget, avail)
            slots_left[0] -= 1
            done = 0
            while queue and done < budget:
                cyc, fn = queue.pop(0)
                queue_cycles[0] -= cyc
                fn()
                done += cyc

        def drain_queue():
            while queue:
                _, fn = queue.pop(0)
                fn()

        # ---- per-image tiles ----
        # x arrives pre-padded + pre-cast to bf16 from the host: contiguous
        # DMA, no border memsets, no on-device casts
        xtiles = {}

        def xload_dma(img):
            xpadb = xpool.tile([128, CC, HP, HP], BF16, tag="xpadb",
                               name=f"xpadb{img}")
            xtiles[img] = (xpadb, xpadb)

            def chunk(cc):
                nc.sync.dma_start(
                    out=xpadb[:, cc].rearrange("p r c -> p (r c)"),
                    in_=x_d[img, cc])
            return chunk

        def xflat(t, cc):  # unpadded [p, 32, 32] view
            return t[:, cc, 1:HP - 1, 1:HP - 1]

        # ---- qkv ----
        qkv_tiles = {}

        def qkv_alloc(img):
            # vaug: [p, mc-pair, pair-slot, head, 48] fp8 (cols 0:32 = v, 32 =
            # ones; 48-stride keeps the DR pair step 16B-aligned)
            d = {
                "q": xpool.tile([128, CC, N], FP8, tag="q", name=f"q{img}"),
                "k": xpool.tile([128, CC, 2, N], FP8, tag="k", name=f"k{img}"),
                "xnb": xpool.tile([128, CC, N], BF16, tag="xnb", name=f"xnb{img}"),
                "vaug": xpool.tile([128, MC // 2, 2, HEADS, 48], FP8, tag="vaug",
                                   name=f"vaug{img}"),
                "attnT": xpool.tile([128, CC, NCH * 128], BF16, tag="attnT",
                                    name=f"attnT{img}"),
                "A": xpool.tile([128, CC, N], BF16, tag="A", name=f"A{img}"),
            }
            # zero second k-tiles for the DoubleRow zero-pad trick (on DVE:
            # Pool's sequencer must stay clear for the startup x casts)
            nc.vector.memset(d["k"][:, :, 1, :], 0.0)
            qkv_tiles[img] = d
            return d

        def qk_chunk(img, oc, which):
            d = qkv_tiles[img]
            xpadb = xtiles[img][1]
            wb = o_q if which == "q" else o_k
            for nh in range(2):
                ps = ps_sc.tile([128, 512], F32, tag="sc",
                                name=f"ps{which}{img}_{oc}_{nh}")
                for ic in range(CC):
                    nc.tensor.matmul(
                        ps,
                        lhsT=wpack[:, wb + ic * C + oc * 128:
                                   wb + ic * C + (oc + 1) * 128],
                        rhs=xflat(xpadb, ic)[:, nh * 16:(nh + 1) * 16, :],
                        start=(ic == 0), stop=(ic == CC - 1))
                if which == "q":
                    nc.scalar.activation(d["q"][:, oc, nh * 512:(nh + 1) * 512],
                                         ps, AF.Identity, bias=qbias(oc))
                else:
                    nc.vector.tensor_copy(d["k"][:, oc, 0, nh * 512:(nh + 1) * 512],
                                          ps)

        def v_chunk(img, half):
            d = qkv_tiles[img]
            xpadb = xtiles[img][1]
            if half == 0:
                nc.gpsimd.memset(d["vaug"][:, :, :, :, D], 1.0)
                for cc in range(CC):
                    nc.gpsimd.tensor_copy(
                        d["xnb"][:, cc].rearrange("p (r c) -> p r c", r=H),
                        xflat(xpadb, cc))
            for pair in range(2):
                ps = ps_sc.tile([128, 512], F32, tag="sc",
                                name=f"psv{img}_{half}_{pair}")
                for sl in range(2):
                    mc = half * 4 + pair * 2 + sl
                    for ic in range(CC):
                        nc.tensor.matmul(
                            ps[:, sl * C:(sl + 1) * C],
                            lhsT=d["xnb"][:, ic, mc * 128:(mc + 1) * 128],
                            rhs=wpack[:, o_v + ic * C: o_v + (ic + 1) * C],
                            start=(ic == 0), stop=(ic == CC - 1))
                for sl in range(2):
                    mc = half * 4 + pair * 2 + sl
                    nc.vector.tensor_copy(
                        d["vaug"][:, mc // 2, mc % 2, :, 0:D],
                        ps[:, sl * C:(sl + 1) * C].rearrange("p (h e) -> p h e",
                                                             h=HEADS))

        # ---- conv chains (filler units) ----
        def push_conv_units(img, cname, w_base, oc, nh):
            state = {}
            mmlist = [(ic, tap) for ic in range(CC) for tap in range(9)]

            def consume(ps):
                xpad, xpadb = xtiles[img]
                if cname == "c1":
                    # on Act: Relu(ps + shift1) — Act has slack since 1/3 of
                    # the exp stream moved to DVE
                    nc.scalar.activation(
                        xflat(c1pads[img], oc)[:, nh * 16:(nh + 1) * 16, :],
                        ps.rearrange("p (r c) -> p r c", r=16),
                        AF.Relu, bias=shift1(oc))
                else:
                    nc.vector.scalar_tensor_tensor(
                        out=c2xs[img][:, oc, nh * 512:(nh + 1) * 512]
                            .rearrange("p (r c) -> p r c", r=16),
                        in0=ps.rearrange("p (r c) -> p r c", r=16),
                        scalar=shiftF(oc),
                        in1=xflat(xpadb, oc)[:, nh * 16:(nh + 1) * 16, :],
                        op0=ALU.add, op1=ALU.add)

            def mk(i0, i1):
                def fn():
                    if "ps" not in state:
                        state["ps"] = ps_cv.tile([128, 512], F32, tag="cv",
                                                 name=f"{cname}{img}_{oc}_{nh}")
                    ps = state["ps"]
                    src = xtiles[img][1] if cname == "c1" else c1pads[img]
                    for idx in range(i0, i1):
                        ic, tap = mmlist[idx]
                        ky, kx = divmod(tap, 3)
                        nc.tensor.matmul(
                            ps,
                            lhsT=conv_w(w_base, ic, tap, oc),
                            rhs=src[:, ic, ky + nh * 16:ky + nh * 16 + 16, kx:kx + W],
                            start=(idx == 0), stop=(idx == 17))
                    if i1 == 18:
                        consume(ps)
                return fn

            for i0 in range(0, 18, 3):
                push(3 * 512, mk(i0, min(i0 + 3, 18)))

        c1pads, c2xs = {}, {}

        def conv_alloc(img):
            c1pad = xpool.tile([128, CC, HP, HP], BF16, tag="c1pad", name=f"c1p{img}")
            for cc in range(CC):
                nc.gpsimd.memset(c1pad[:, cc, 0, :], 0.0)
                nc.gpsimd.memset(c1pad[:, cc, HP - 1, :], 0.0)
                nc.gpsimd.memset(c1pad[:, cc, 1:HP - 1, 0], 0.0)
                nc.gpsimd.memset(c1pad[:, cc, 1:HP - 1, HP - 1], 0.0)
            c1pads[img] = c1pad
            c2xs[img] = xpool.tile([128, CC, N], F32, tag="c2x", name=f"c2x{img}")

        def push_conv_all(img):
            for oc in range(CC):
                for nh in range(2):
                    push_conv_units(img, "c1", o_w1, oc, nh)
            for oc in range(CC):
                for nh in range(2):
                    push_conv_units(img, "c2", o_w2, oc, nh)

        # ---- attention ----
        def emit_head_norm(img, h, at):
            # one broadcast multiply per head: out[p,g,c] = at[p,g,c]*rcp[p,g]
            d = qkv_tiles[img]
            rcp = xpool.tile([128, NCH], F32, tag="rcp", name=f"rcp{img}_{h}")
            nc.vector.reciprocal(
                rcp, at.rearrange("p (g e) -> p g e", e=DAUG)[:, :, D])
            rcp_bc = bass.AP(tensor=rcp.tensor, offset=rcp.offset,
                             ap=[list(rcp.ap[0])] + [[1, NCH], [0, D]])
            cch, hh = h // 4, h % 4
            nc.vector.scalar_tensor_tensor(
                out=d["attnT"][:, cch].rearrange("p (g c) -> p g c", c=128)
                    [:, :, hh * D:(hh + 1) * D],
                in0=at.rearrange("p (g e) -> p g e", e=DAUG)[:, :, 0:D],
                scalar=0.0,
                in1=rcp_bc,
                op0=ALU.add, op1=ALU.mult)

        def push_transp(img, cc):
            # batched DMA XBAR transpose: ONE instruction flips all 8 128x128
            # blocks of a cc-half (SBUF->SBUF, no PSUM)
            d = qkv_tiles[img]

            # emitted IMMEDIATELY at the trigger point (not queued): proj fns
            # are front-pushed and would otherwise emit before this transpose,
            # reading A before it is written (no dep in the Tile trace)
            if img == IMGS - 1 and cc == 1:
                # tail: PE-mode transpose + Act copy -- PE and Act are idle
                # here, and this skips the ~2.4us DMA/DGE latency chain
                ps = ps_sc.tile([128, N], BF16, tag="sc", name="trtail")
                for b in range(NCH):
                    nc.tensor.transpose(
                        ps[:, b * 128:(b + 1) * 128],
                        d["attnT"][:, cc, b * 128:(b + 1) * 128],
                        wpack[:, O_IDENT:O_IDENT + 128])
                nc.scalar.activation(d["A"][:, cc], ps, AF.Identity)
            else:
                nc.sync.dma_start_transpose(
                    out=d["A"][:, cc].rearrange("p (a b) -> p a b", a=NCH),
                    in_=d["attnT"][:, cc])

        def push_proj(img):
            d = qkv_tiles[img]
            for oc in range(CC):
                for nh in range(2):
                    def fn(oc=oc, nh=nh):
                        # img1's proj runs at the tail when the score banks are
                        # free: 4-deep rotation lets all 4 groups pipeline
                        pool = ps_sc if img == IMGS - 1 else ps_cv
                        pj = pool.tile([128, 512], F32, tag="sc" if img == IMGS - 1 else "cv",
                                       name=f"pj{img}_{oc}_{nh}")
                        for cc in range(CC):
                            nc.tensor.matmul(
                                pj,
                                lhsT=wpack[:, o_ow + cc * C + oc * 128:
                                           o_ow + cc * C + oc * 128 + 128],
                                rhs=d["A"][:, cc, nh * 512:(nh + 1) * 512],
                                start=(cc == 0), stop=(cc == CC - 1))
                        cmb = xpool.tile([128, 512], F32, tag="cmb",
                                         name=f"cmb{img}_{oc}_{nh}")
                        nc.vector.scalar_tensor_tensor(
                            out=cmb, in0=pj, scalar=0.0,
                            in1=c2xs[img][:, oc, nh * 512:(nh + 1) * 512],
                            op0=ALU.add, op1=ALU.add)
                        osb = xpool.tile([128, 512], F32, tag="osb",
                                         name=f"osb{img}_{oc}_{nh}")
                        if img == IMGS - 1:
                            # tail: Act is idle after the last exp
                            nc.scalar.activation(osb, cmb, AF.Relu)
                        else:
                            nc.gpsimd.tensor_scalar(osb, cmb, 0.0, None, ALU.max)
                        nc.sync.dma_start(
                            out=out_d[img, oc, :, nh * 512:(nh + 1) * 512], in_=osb)
                    # img0's proj flows as immediate filler; img1's drains at
                    # the tail after the conv reserve covers transpose latency
                    push(1024, fn, front=(img == 0))

        # attention sub-blocks: one (head, ncb) group per slot, lagged one
        # full head so all 8 pt tiles of the head exist. Groups are strictly
        # SEQUENTIAL within the psum bank (ncb outer, mc inner): TRN2's
        # start=True lazily re-arms the whole 2KB zero region, so interleaved
        # per-group starts would wipe other groups' partial accumulations.
        attn_subs = []
        at_tiles = {}

        def make_attn_subs(img, h, pts):
            d = qkv_tiles[img]

            def mk(ncb):
                def fn():
                    if ncb == 0:
                        at_tiles[(img, h)] = ps_at.tile(
                            [128, NCH * DAUG], F32, tag="at", name=f"at{img}_{h}")
                    at = at_tiles[(img, h)]
                    for pr in range(MC // 2):
                        # DoubleRow over a REAL mc-pair: pt pair tile is the
                        # lhsT k-pair, vaug pair-slots are the rhs
                        nc.tensor.matmul(
                            at[:, ncb * DAUG:(ncb + 1) * DAUG],
                            lhsT=pts[pr][:, :, ncb * 128:(ncb + 1) * 128],
                            rhs=d["vaug"][:, pr, :, h, 0:DAUG],
                            start=(pr == 0), stop=(pr == MC // 2 - 1),
                            perf_mode=DR, skip_group_check=True)
                    if ncb == NCH - 1:
                        emit_head_norm(img, h, at)
                        del at_tiles[(img, h)]
                        if h == 3:
                            push_transp(img, 0)
                        if h == 7:
                            push_transp(img, 1)
                            push_proj(img)
                return fn

            for ncb in range(NCH):
                attn_subs.append(mk(ncb))

        def head_loop(img):
            d = qkv_tiles[img]
            q_sb, k_sb = d["q"], d["k"]
            for h in range(HEADS):
                hp, cch = 32 * (h % 4), h // 4
                pts = []
                for mc in range(MC):
                    if mc % 2 == 0:
                        pts.append(ptpool.tile([128, 2, N], FP8, tag="pt",
                                               name=f"pt{img}_{h}_{mc // 2}"))
                    for nh in range(2):
                        # half-tile score psum (one bank) -> 4-deep rotation so
                        # the exp latency stays off the matmul critical path
                        sc = ps_sc.tile([128, 512], F32, tag="sc",
                                        name=f"sc{img}_{h}_{mc}_{nh}")
                        nc.tensor.matmul(
                            sc,
                            lhsT=k_sb[hp:hp + 32, cch, :, mc * 128:(mc + 1) * 128],
                            rhs=_dr0(q_sb[hp:hp + 32, cch,
                                          nh * 512:(nh + 1) * 512]),
                            start=True, stop=True, perf_mode=DR,
                            tile_position=(hp, 0))
                        slot = pts[-1][:, mc % 2, nh * 512:(nh + 1) * 512]
                        idx = ((img * HEADS + h) * MC + mc) * 2 + nh
                        if idx % EXP_MOD < EXP_DVE:
                            # fp8e4 Schraudolph on DVE: linear-bits in uint8;
                            # saturation maps s-4 < -10.4 to p=0 (tiny mass)
                            nc.vector.tensor_scalar(
                                slot.bitcast(mybir.dt.uint8), sc,
                                E4_MULT, E4_ADD, ALU.mult, ALU.add)
                        else:
                            nc.scalar.activation(slot, sc, AF.Exp, bias=expshift)
                    if attn_subs:
                        attn_subs.pop(0)()
                    pop_fill()
                make_attn_subs(img, h, pts)

        # ================= emission =================
        # DMA order matters: transfers serialize on the DMA engines, so the
        # small qkv weights + vecs go first (first consumers), then image 0,
        # then the two conv-weight halves, then image 1.
        chunk0 = xload_dma(0)
        nc.sync.dma_start(out=wpack[:, o_q:], in_=wp_d[:, o_q:])
        chunk0(0)
        chunk0(1)
        nc.sync.dma_start(out=vecs, in_=vec_d[:])
        nc.sync.dma_start(out=wpack[:, :W1_COLS], in_=wp_d[:, :W1_COLS])
        nc.sync.dma_start(out=wpack[:, W1_COLS:o_q], in_=wp_d[:, W1_COLS:o_q])
        chunk1 = xload_dma(1)
        chunk1(0)
        chunk1(1)

        qkv_alloc(0)
        qk_chunk(0, 0, "q")
        qk_chunk(0, 0, "k")
        push(0, lambda: conv_alloc(0))
        push(2048, lambda: v_chunk(0, 0), front=True)
        push(2048, lambda: v_chunk(0, 1))
        push(2048, lambda: qk_chunk(0, 1, "q"))
        push(2048, lambda: qk_chunk(0, 1, "k"))
        push_conv_all(0)
        # image 1 prep as filler inside image 0's slots
        push(0, lambda: (qkv_alloc(1), conv_alloc(1)) and None)
        push(2048, lambda: qk_chunk(1, 0, "q"))
        push(2048, lambda: qk_chunk(1, 0, "k"))
        push(2048, lambda: v_chunk(1, 0))
        push(2048, lambda: v_chunk(1, 1))
        push(2048, lambda: qk_chunk(1, 1, "q"))
        push(2048, lambda: qk_chunk(1, 1, "k"))

        head_loop(0)
        push_conv_all(1)
        head_loop(1)
        while attn_subs:
            attn_subs.pop(0)()
        drain_queue()
        if KDBG:
            for img in range(IMGS):
                d = qkv_tiles[img]
                for cc in range(CC):
                    nc.sync.dma_start(out=dbgA[img, cc], in_=d["A"][:, cc])
                    nc.sync.dma_start(out=dbgT[img, cc], in_=d["attnT"][:, cc])
                    nc.sync.dma_start(out=dbgC[img, cc], in_=c2xs[img][:, cc])

    nc.finalize()
    return nc


def _prep_inputs(inputs: dict) -> list[dict]:
    bf = ml_dtypes.bfloat16
    x = np.asarray(inputs["x"], dtype=np.float32)
    f32 = lambda k: np.asarray(inputs[k], dtype=np.float32)
    bn1_inv = f32("bn1_gamma") / np.sqrt(f32("bn1_var") + EPS)
    shift1 = f32("bn1_beta") - f32("bn1_mean") * bn1_inv + f32("conv1_b") * bn1_inv
    w1s = f32("conv1_w") * bn1_inv[:, None, None, None]
    bn2_inv = f32("bn2_gamma") / np.sqrt(f32("bn2_var") + EPS)
    shift2 = f32("bn2_beta") - f32("bn2_mean") * bn2_inv + f32("conv2_b") * bn2_inv
    w2s = f32("conv2_w") * bn2_inv[:, None, None, None]
    sg = 1.0 / (1.0 + np.exp(-float(np.asarray(inputs["gate"]))))
    ow = f32("out_w") * sg
    shiftF = shift2 + sg * f32("out_b") + sg * (f32("out_w") @ f32("v_b"))
    qws = f32("q_w") / np.sqrt(D)
    qbs = f32("q_b") / np.sqrt(D)

    def conv_pack(w):  # [O, I, 3, 3] -> [128, CC*9*CC*128]
        t = w.transpose(1, 2, 3, 0).reshape(CC, 128, 3, 3, CC, 128)
        return t.transpose(1, 0, 2, 3, 4, 5).reshape(128, W1_COLS)

    def pack_T(w):  # [O, C_in] -> [128, CC*C]
        return w.T.reshape(CC, 128, C).transpose(1, 0, 2).reshape(128, QKVO_COLS)

    wpack = np.concatenate(
        [conv_pack(w1s), conv_pack(w2s), pack_T(qws), pack_T(f32("k_w")),
         pack_T(f32("v_w")), pack_T(ow), np.eye(128, dtype=np.float32)],
        axis=1).astype(bf)
    assert wpack.shape == (128, PACK_COLS)

    vecs = np.concatenate(
        [np.stack([shift1.reshape(CC, 128), shiftF.reshape(CC, 128),
                   qbs.reshape(CC, 128)]).reshape(3 * CC, 128).T,
         np.eye(128, dtype=np.float32),
         np.full((128, 1), EXP_SHIFT, np.float32)], axis=1)
    assert vecs.shape == (128, VEC_COLS)
    shared = {"wpack": np.ascontiguousarray(wpack),
              "vecs": np.ascontiguousarray(vecs.astype(np.float32))}
    # pre-pad + pre-cast x to bf16: [B, CC, 128, HP, HP] with zero borders
    xp = np.zeros((B, CC, 128, HP, HP), dtype=bf)
    xp[:, :, :, 1:HP - 1, 1:HP - 1] = (
        x.reshape(B, CC, 128, H, W).astype(bf))
    xp = xp.reshape(B, CC, 128, HP * HP)
    in_maps = []
    for core in range(N_CORES):
        xs = xp[core * IMGS:(core + 1) * IMGS]
        in_maps.append({"x_sh": np.ascontiguousarray(xs), **shared})
    return in_maps


_NC_CACHE = {}


def _get_nc():
    if "nc" not in _NC_CACHE:
        _NC_CACHE["nc"] = build_nc()
    return _NC_CACHE["nc"]


def kernel(**inputs) -> np.ndarray:
    nc = _get_nc()
    in_maps = _prep_inputs(inputs)
    res = run_bass_kernel_spmd(nc, in_maps, core_ids=list(range(N_CORES)))
    outs = [res.results[i]["out_sh"].reshape(IMGS, C, H, W) for i in range(N_CORES)]
    return np.concatenate(outs, axis=0)



# revision 43
# speedup vs baseline: 1.0064x; 1.0023x over previous
"""Trainium2 Bass kernel for nn_AttentionResidualBlock (B=16, C=256, H=W=32, heads=8).

Sharding: data-parallel over batch across 8 NeuronCores (2 images/core),
weights replicated.

Per core (per image):
  - conv3x3 as 9 shifted bf16 matmuls over a zero-padded [C, 34, 34] layout;
    BN scale folded into weights on host, BN shift + ReLU fused on DVE.
    Conv work paces as TensorE filler inside the attention slots.
  - attention head-by-head: scoresT[m,n] = k^T q via fp8e4 DoubleRow
    matmuls (k carries a zeroed second k-tile; both operands fp8) at half
    the bf16 streaming cost; exp on ScalarE (PSUM -> SBUF bf16); attn@v
    computed TRANSPOSED: out[n, d] = sum_m pt[m, n] v[m, d] with
    lhsT = pt blocks and a ones-augmented v (33 cols) so the softmax
    denominator lands on the same partition as its outputs. Normalize is
    a per-partition reciprocal + tensor_scalar multiply. The 8 psum
    accumulation groups per head run ncb-outer/mc-inner (sequential per
    bank) because start=True lazily re-arms the whole 2KB zero region.
  - attnT is transposed back to [c, n] with DMA XBAR transposes
    (SBUF->SBUF, no PSUM), then a dense out-projection; gate/out-bias/
    v-bias folded on host.
Conv/attn@v/proj matmuls bf16, scores fp8-DR, fp32 PSUM accumulation.
"""

import os
import numpy as np
import ml_dtypes
from contextlib import ExitStack

KDBG = bool(int(os.environ.get("KDBG", "0")))

import concourse.bass as bass
import concourse.bacc as bacc
import concourse.mybir as mybir
import concourse.tile as tile
from concourse.bass_utils import run_bass_kernel_spmd

F32 = mybir.dt.float32
BF16 = mybir.dt.bfloat16
FP8 = mybir.dt.float8e4
AF = mybir.ActivationFunctionType
ALU = mybir.AluOpType
DR = mybir.MatmulPerfMode.DoubleRow
EXP_SHIFT = -4.0  # exp(s-4): keeps fp8e4 probabilities in range
# fp8e4 Schraudolph constants for exp(s-4): bits = s*11.5416 + 9.4896, with
# uint8 saturation mapping s < -0.82 to p=0 (verified on HW; negligible mass)
E4_MULT, E4_ADD = 11.5416, 56.0 - 4 * 11.5416 - 0.344
# exp slots alternate DVE (uint8 Schraudolph) / Act (real exp -> fp8):
# idx % EXP_MOD < EXP_DVE go to DVE
EXP_MOD, EXP_DVE = 5, 2


def _dr0(ap: bass.AP) -> bass.AP:
    """Insert a stride-0 dim after the partition dim (DoubleRow k-tile reuse)."""
    return bass.AP(tensor=ap.tensor, offset=ap.offset,
                   ap=[list(ap.ap[0])] + [[0, 2]] + [list(d) for d in ap.ap[1:]])

C = 256
HEADS = 8
D = 32
B, H, W = 16, 32, 32
N = H * W          # 1024
HP = H + 2         # 34
EPS = 1e-5
N_CORES = 8
IMGS = B // N_CORES  # 2 images per core
CC = C // 128      # 2 channel chunks
MC = N // 128      # 8 spatial m-chunks
NCH = 8            # n-chunks for attn output
DAUG = D + 1       # 33 (v cols + ones col)

# packed bf16 weight layout (columns per partition)
W1_COLS = CC * 9 * CC * 128          # 4608
QKVO_COLS = CC * C                   # 512
PACK_COLS = 2 * W1_COLS + 4 * QKVO_COLS + 128  # w1 w2 q k v ow ident = 11392
O_IDENT = PACK_COLS - 128            # bf16 identity for PE-mode transpose
VEC_COLS = 3 * CC + 128 + 1          # shift1, shiftF, qbias, identity(f32), exp-shift


def build_nc() -> bass.Bass:
    nc = bacc.Bacc()

    x_d = nc.declare_dram_parameter("x_sh", [IMGS, CC, 128, HP * HP], BF16,
                                    isOutput=False)
    wp_d = nc.declare_dram_parameter("wpack", [128, PACK_COLS], BF16, isOutput=False)
    vec_d = nc.declare_dram_parameter("vecs", [128, VEC_COLS], F32, isOutput=False)
    out_d = nc.declare_dram_parameter("out_sh", [IMGS, CC, 128, N], F32, isOutput=True)
    if KDBG:
        dbgA = nc.declare_dram_parameter("dbg_A", [IMGS, CC, 128, N], BF16,
                                         isOutput=True)
        dbgT = nc.declare_dram_parameter("dbg_attnT", [IMGS, CC, 128, N], BF16,
                                         isOutput=True)
        dbgC = nc.declare_dram_parameter("dbg_c2x", [IMGS, CC, 128, N], F32,
                                         isOutput=True)

    o_w1, o_w2 = 0, W1_COLS
    o_q = 2 * W1_COLS
    o_k, o_v = o_q + QKVO_COLS, o_q + 2 * QKVO_COLS
    o_ow = o_q + 3 * QKVO_COLS

    with ExitStack() as ctx:
        tc = ctx.enter_context(tile.TileContext(nc))
        wpool = ctx.enter_context(tc.tile_pool(name="weights", bufs=1))
        xpool = ctx.enter_context(tc.tile_pool(name="acts", bufs=2))
        ptpool = ctx.enter_context(tc.tile_pool(name="pt", bufs=10))
        ps_sc = ctx.enter_context(tc.tile_pool(name="ps_sc", bufs=4, space="PSUM"))
        ps_at = ctx.enter_context(tc.tile_pool(name="ps_at", bufs=2, space="PSUM"))
        ps_cv = ctx.enter_context(tc.tile_pool(name="ps_cv", bufs=2, space="PSUM"))

        # ---- weights / vectors ----
        wpack = wpool.tile([128, PACK_COLS], BF16, tag="wpack")
        vecs = wpool.tile([128, VEC_COLS], F32, tag="vecs")

        def conv_w(base, ic, tap, oc):  # [128, 128] lhsT slice
            off = base + ((ic * 9 + tap) * CC + oc) * 128
            return wpack[:, off:off + 128]

        shift1 = lambda oc: vecs[:, oc:oc + 1]
        shiftF = lambda oc: vecs[:, CC + oc:CC + oc + 1]
        qbias = lambda oc: vecs[:, 2 * CC + oc:2 * CC + oc + 1]
        ident = vecs[:, 3 * CC:3 * CC + 128]
        expshift = vecs[:, 3 * CC + 128:3 * CC + 129]

        # ---- filler queue (PE work units paced into attention slots) ----
        queue = []

        def push(cycles, fn, front=False):
            queue_cycles[0] += cycles
            if front:
                queue.insert(0, (cycles, fn))
            else:
                queue.append((cycles, fn))

        queue_cycles = [0]   # running total of cycles in queue
        slots_left = [128]   # attention slots remaining in the whole program

        RESERVE = 7000  # PE cycles held back to fill the post-last-exp tail

        def pop_fill():
            # spread remaining queue work evenly over remaining slots, but
            # never burst past the Act period (starves the exp pacer) nor
            # drip so slowly that conv debt piles up past the last exp; keep
            # RESERVE cycles back so the tail chain (norm/transpose/proj)
            # overlaps PE work instead of idling it
            avail = max(0, queue_cycles[0] - RESERVE)
            budget = min(1600, max(1200, avail // max(1, slots_left[0])))
            budget = min(budget, avail)
            slots_left[0] -= 1
            done = 0
            while queue and done < budget:
                cyc, fn = queue.pop(0)
                queue_cycles[0] -= cyc
                fn()
                done += cyc

        def drain_queue():
            while queue:
                _, fn = queue.pop(0)
                fn()

        # ---- per-image tiles ----
        # x arrives pre-padded + pre-cast to bf16 from the host: contiguous
        # DMA, no border memsets, no on-device casts
        xtiles = {}

        def xload_dma(img):
            xpadb = xpool.tile([128, CC, HP, HP], BF16, tag="xpadb",
                               name=f"xpadb{img}")
            xtiles[img] = (xpadb, xpadb)

            def chunk(cc):
                nc.sync.dma_start(
                    out=xpadb[:, cc].rearrange("p r c -> p (r c)"),
                    in_=x_d[img, cc])
            return chunk

        def xflat(t, cc):  # unpadded [p, 32, 32] view
            return t[:, cc, 1:HP - 1, 1:HP - 1]

        # ---- qkv ----
        qkv_tiles = {}

        def qkv_alloc(img):
            # vaug: [p, mc-pair, pair-slot, head, 48] fp8 (cols 0:32 = v, 32 =
            # ones; 48-stride keeps the DR pair step 16B-aligned)
            d = {
                "q": xpool.tile([128, CC, N], FP8, tag="q", name=f"q{img}"),
                "k": xpool.tile([128, CC, 2, N], FP8, tag="k", name=f"k{img}"),
                "xnb": xpool.tile([128, CC, N], BF16, tag="xnb", name=f"xnb{img}"),
                "vaug": xpool.tile([128, MC // 2, 2, HEADS, 48], FP8, tag="vaug",
                                   name=f"vaug{img}"),
                "attnT": xpool.tile([128, CC, NCH * 128], BF16, tag="attnT",
                                    name=f"attnT{img}"),
                "A": xpool.tile([128, CC, N], BF16, tag="A", name=f"A{img}"),
            }
            # zero second k-tiles for the DoubleRow zero-pad trick (on DVE:
            # Pool's sequencer must stay clear for the startup x casts)
            nc.vector.memset(d["k"][:, :, 1, :], 0.0)
            qkv_tiles[img] = d
            return d

        def qk_chunk(img, oc, which):
            d = qkv_tiles[img]
            xpadb = xtiles[img][1]
            wb = o_q if which == "q" else o_k
            for nh in range(2):
                ps = ps_sc.tile([128, 512], F32, tag="sc",
                                name=f"ps{which}{img}_{oc}_{nh}")
                for ic in range(CC):
                    nc.tensor.matmul(
                        ps,
                        lhsT=wpack[:, wb + ic * C + oc * 128:
                                   wb + ic * C + (oc + 1) * 128],
                        rhs=xflat(xpadb, ic)[:, nh * 16:(nh + 1) * 16, :],
                        start=(ic == 0), stop=(ic == CC - 1))
                if which == "q":
                    nc.scalar.activation(d["q"][:, oc, nh * 512:(nh + 1) * 512],
                                         ps, AF.Identity, bias=qbias(oc))
                else:
                    nc.vector.tensor_copy(d["k"][:, oc, 0, nh * 512:(nh + 1) * 512],
                                          ps)

        def v_chunk(img, half):
            d = qkv_tiles[img]
            xpadb = xtiles[img][1]
            if half == 0:
                nc.gpsimd.memset(d["vaug"][:, :, :, :, D], 1.0)
                for cc in range(CC):
                    nc.gpsimd.tensor_copy(
                        d["xnb"][:, cc].rearrange("p (r c) -> p r c", r=H),
                        xflat(xpadb, cc))
            for pair in range(2):
                ps = ps_sc.tile([128, 512], F32, tag="sc",
                                name=f"psv{img}_{half}_{pair}")
                for sl in range(2):
                    mc = half * 4 + pair * 2 + sl
                    for ic in range(CC):
                        nc.tensor.matmul(
                            ps[:, sl * C:(sl + 1) * C],
                            lhsT=d["xnb"][:, ic, mc * 128:(mc + 1) * 128],
                            rhs=wpack[:, o_v + ic * C: o_v + (ic + 1) * C],
                            start=(ic == 0), stop=(ic == CC - 1))
                for sl in range(2):
                    mc = half * 4 + pair * 2 + sl
                    nc.vector.tensor_copy(
                        d["vaug"][:, mc // 2, mc % 2, :, 0:D],
                        ps[:, sl * C:(sl + 1) * C].rearrange("p (h e) -> p h e",
                                                             h=HEADS))

        # ---- conv chains (filler units) ----
        def push_conv_units(img, cname, w_base, oc, nh):
            state = {}
            mmlist = [(ic, tap) for ic in range(CC) for tap in range(9)]

            def consume(ps):
                xpad, xpadb = xtiles[img]
                if cname == "c1":
                    # on Act: Relu(ps + shift1) — Act has slack since 1/3 of
                    # the exp stream moved to DVE
                    nc.scalar.activation(
                        xflat(c1pads[img], oc)[:, nh * 16:(nh + 1) * 16, :],
                        ps.rearrange("p (r c) -> p r c", r=16),
                        AF.Relu, bias=shift1(oc))
                else:
                    nc.vector.scalar_tensor_tensor(
                        out=c2xs[img][:, oc, nh * 512:(nh + 1) * 512]
                            .rearrange("p (r c) -> p r c", r=16),
                        in0=ps.rearrange("p (r c) -> p r c", r=16),
                        scalar=shiftF(oc),
                        in1=xflat(xpadb, oc)[:, nh * 16:(nh + 1) * 16, :],
                        op0=ALU.add, op1=ALU.add)

            def mk(i0, i1):
                def fn():
                    if "ps" not in state:
                        state["ps"] = ps_cv.tile([128, 512], F32, tag="cv",
                                                 name=f"{cname}{img}_{oc}_{nh}")
                    ps = state["ps"]
                    src = xtiles[img][1] if cname == "c1" else c1pads[img]
                    for idx in range(i0, i1):
                        ic, tap = mmlist[idx]
                        ky, kx = divmod(tap, 3)
                        nc.tensor.matmul(
                            ps,
                            lhsT=conv_w(w_base, ic, tap, oc),
                            rhs=src[:, ic, ky + nh * 16:ky + nh * 16 + 16, kx:kx + W],
                            start=(idx == 0), stop=(idx == 17))
                    if i1 == 18:
                        consume(ps)
                return fn

            for i0 in range(0, 18, 3):
                push(3 * 512, mk(i0, min(i0 + 3, 18)))

        c1pads, c2xs = {}, {}

        def conv_alloc(img):
            c1pad = xpool.tile([128, CC, HP, HP], BF16, tag="c1pad", name=f"c1p{img}")
            for cc in range(CC):
                nc.gpsimd.memset(c1pad[:, cc, 0, :], 0.0)
                nc.gpsimd.memset(c1pad[:, cc, HP - 1, :], 0.0)
                nc.gpsimd.memset(c1pad[:, cc, 1:HP - 1, 0], 0.0)
                nc.gpsimd.memset(c1pad[:, cc, 1:HP - 1, HP - 1], 0.0)
            c1pads[img] = c1pad
            c2xs[img] = xpool.tile([128, CC, N], F32, tag="c2x", name=f"c2x{img}")

        def push_conv_all(img):
            for oc in range(CC):
                for nh in range(2):
                    push_conv_units(img, "c1", o_w1, oc, nh)
            for oc in range(CC):
                for nh in range(2):
                    push_conv_units(img, "c2", o_w2, oc, nh)

        # ---- attention ----
        def emit_head_norm(img, h, at):
            # one broadcast multiply per head: out[p,g,c] = at[p,g,c]*rcp[p,g]
            d = qkv_tiles[img]
            rcp = xpool.tile([128, NCH], F32, tag="rcp", name=f"rcp{img}_{h}")
            nc.vector.reciprocal(
                rcp, at.rearrange("p (g e) -> p g e", e=DAUG)[:, :, D])
            rcp_bc = bass.AP(tensor=rcp.tensor, offset=rcp.offset,
                             ap=[list(rcp.ap[0])] + [[1, NCH], [0, D]])
            cch, hh = h // 4, h % 4
            nc.vector.scalar_tensor_tensor(
                out=d["attnT"][:, cch].rearrange("p (g c) -> p g c", c=128)
                    [:, :, hh * D:(hh + 1) * D],
                in0=at.rearrange("p (g e) -> p g e", e=DAUG)[:, :, 0:D],
                scalar=0.0,
                in1=rcp_bc,
                op0=ALU.add, op1=ALU.mult)

        def push_transp(img, cc):
            # batched DMA XBAR transpose: ONE instruction flips all 8 128x128
            # blocks of a cc-half (SBUF->SBUF, no PSUM)
            d = qkv_tiles[img]

            # emitted IMMEDIATELY at the trigger point (not queued): proj fns
            # are front-pushed and would otherwise emit before this transpose,
            # reading A before it is written (no dep in the Tile trace)
            if img == IMGS - 1 and cc == 1:
                # tail: PE-mode transpose + Act copy -- PE and Act are idle
                # here, and this skips the ~2.4us DMA/DGE latency chain
                ps = ps_sc.tile([128, N], BF16, tag="sc", name="trtail")
                for b in range(NCH):
                    nc.tensor.transpose(
                        ps[:, b * 128:(b + 1) * 128],
                        d["attnT"][:, cc, b * 128:(b + 1) * 128],
                        wpack[:, O_IDENT:O_IDENT + 128])
                nc.vector.tensor_copy(d["A"][:, cc], ps)
            else:
                nc.sync.dma_start_transpose(
                    out=d["A"][:, cc].rearrange("p (a b) -> p a b", a=NCH),
                    in_=d["attnT"][:, cc])

        def push_proj(img):
            d = qkv_tiles[img]
            for oc in range(CC):
                for nh in range(2):
                    def fn(oc=oc, nh=nh):
                        # img1's proj runs at the tail when the score banks are
                        # free: 4-deep rotation lets all 4 groups pipeline
                        pool = ps_sc if img == IMGS - 1 else ps_cv
                        pj = pool.tile([128, 512], F32, tag="sc" if img == IMGS - 1 else "cv",
                                       name=f"pj{img}_{oc}_{nh}")
                        for cc in range(CC):
                            nc.tensor.matmul(
                                pj,
                                lhsT=wpack[:, o_ow + cc * C + oc * 128:
                                           o_ow + cc * C + oc * 128 + 128],
                                rhs=d["A"][:, cc, nh * 512:(nh + 1) * 512],
                                start=(cc == 0), stop=(cc == CC - 1))
                        cmb = xpool.tile([128, 512], F32, tag="cmb",
                                         name=f"cmb{img}_{oc}_{nh}")
                        nc.vector.scalar_tensor_tensor(
                            out=cmb, in0=pj, scalar=0.0,
                            in1=c2xs[img][:, oc, nh * 512:(nh + 1) * 512],
                            op0=ALU.add, op1=ALU.add)
                        osb = xpool.tile([128, 512], F32, tag="osb",
                                         name=f"osb{img}_{oc}_{nh}")
                        nc.gpsimd.tensor_scalar(osb, cmb, 0.0, None, ALU.max)
                        nc.sync.dma_start(
                            out=out_d[img, oc, :, nh * 512:(nh + 1) * 512], in_=osb)
                    # img0's proj flows as immediate filler; img1's drains at
                    # the tail after the conv reserve covers transpose latency
                    push(1024, fn, front=(img == 0))

        # attention sub-blocks: one (head, ncb) group per slot, lagged one
        # full head so all 8 pt tiles of the head exist. Groups are strictly
        # SEQUENTIAL within the psum bank (ncb outer, mc inner): TRN2's
        # start=True lazily re-arms the whole 2KB zero region, so interleaved
        # per-group starts would wipe other groups' partial accumulations.
        attn_subs = []
        at_tiles = {}

        def make_attn_subs(img, h, pts):
            d = qkv_tiles[img]

            def mk(ncb):
                def fn():
                    if ncb == 0:
                        at_tiles[(img, h)] = ps_at.tile(
                            [128, NCH * DAUG], F32, tag="at", name=f"at{img}_{h}")
                    at = at_tiles[(img, h)]
                    for pr in range(MC // 2):
                        # DoubleRow over a REAL mc-pair: pt pair tile is the
                        # lhsT k-pair, vaug pair-slots are the rhs
                        nc.tensor.matmul(
                            at[:, ncb * DAUG:(ncb + 1) * DAUG],
                            lhsT=pts[pr][:, :, ncb * 128:(ncb + 1) * 128],
                            rhs=d["vaug"][:, pr, :, h, 0:DAUG],
                            start=(pr == 0), stop=(pr == MC // 2 - 1),
                            perf_mode=DR, skip_group_check=True)
                    if ncb == NCH - 1:
                        emit_head_norm(img, h, at)
                        del at_tiles[(img, h)]
                        if h == 3:
                            push_transp(img, 0)
                        if h == 7:
                            push_transp(img, 1)
                            push_proj(img)
                return fn

            for ncb in range(NCH):
                attn_subs.append(mk(ncb))

        def head_loop(img):
            d = qkv_tiles[img]
            q_sb, k_sb = d["q"], d["k"]
            for h in range(HEADS):
                hp, cch = 32 * (h % 4), h // 4
                pts = []
                for mc in range(MC):
                    if mc % 2 == 0:
                        pts.append(ptpool.tile([128, 2, N], FP8, tag="pt",
                                               name=f"pt{img}_{h}_{mc // 2}"))
                    for nh in range(2):
                        # half-tile score psum (one bank) -> 4-deep rotation so
                        # the exp latency stays off the matmul critical path
                        sc = ps_sc.tile([128, 512], F32, tag="sc",
                                        name=f"sc{img}_{h}_{mc}_{nh}")
                        nc.tensor.matmul(
                            sc,
                            lhsT=k_sb[hp:hp + 32, cch, :, mc * 128:(mc + 1) * 128],
                            rhs=_dr0(q_sb[hp:hp + 32, cch,
                                          nh * 512:(nh + 1) * 512]),
                            start=True, stop=True, perf_mode=DR,
                            tile_position=(hp, 0))
                        slot = pts[-1][:, mc % 2, nh * 512:(nh + 1) * 512]
                        idx = ((img * HEADS + h) * MC + mc) * 2 + nh
                        if idx % EXP_MOD < EXP_DVE:
                            # fp8e4 Schraudolph on DVE: linear-bits in uint8;
                            # saturation maps s-4 < -10.4 to p=0 (tiny mass)
                            nc.vector.tensor_scalar(
                                slot.bitcast(mybir.dt.uint8), sc,
                                E4_MULT, E4_ADD, ALU.mult, ALU.add)
                        else:
                            nc.scalar.activation(slot, sc, AF.Exp, bias=expshift)
                    if attn_subs:
                        attn_subs.pop(0)()
                    pop_fill()
                make_attn_subs(img, h, pts)

        # ================= emission =================
        # DMA order matters: transfers serialize on the DMA engines, so the
        # small qkv weights + vecs go first (first consumers), then image 0,
        # then the two conv-weight halves, then image 1.
        chunk0 = xload_dma(0)
        nc.sync.dma_start(out=wpack[:, o_q:], in_=wp_d[:, o_q:])
        chunk0(0)
        chunk0(1)
        nc.sync.dma_start(out=vecs, in_=vec_d[:])
        nc.sync.dma_start(out=wpack[:, :W1_COLS], in_=wp_d[:, :W1_COLS])
        nc.sync.dma_start(out=wpack[:, W1_COLS:o_q], in_=wp_d[:, W1_COLS:o_q])
        chunk1 = xload_dma(1)
        chunk1(0)
        chunk1(1)

        qkv_alloc(0)
        qk_chunk(0, 0, "q")
        qk_chunk(0, 0, "k")
        push(0, lambda: conv_alloc(0))
        push(2048, lambda: v_chunk(0, 0), front=True)
        push(2048, lambda: v_chunk(0, 1))
        push(2048, lambda: qk_chunk(0, 1, "q"))
        push(2048, lambda: qk_chunk(0, 1, "k"))
        push_conv_all(0)
        # image 1 prep as filler inside image 0's slots
        push(0, lambda: (qkv_alloc(1), conv_alloc(1)) and None)
        push(2048, lambda: qk_chunk(1, 0, "q"))
        push(2048, lambda: qk_chunk(1, 0, "k"))
        push(2048, lambda: v_chunk(1, 0))
        push(2048, lambda: v_chunk(1, 1))
        push(2048, lambda: qk_chunk(1, 1, "q"))
        push(2048, lambda: qk_chunk(1, 1, "k"))

        head_loop(0)
        push_conv_all(1)
        head_loop(1)
        while attn_subs:
            attn_subs.pop(0)()
        drain_queue()
        if KDBG:
            for img in range(IMGS):
                d = qkv_tiles[img]
                for cc in range(CC):
                    nc.sync.dma_start(out=dbgA[img, cc], in_=d["A"][:, cc])
                    nc.sync.dma_start(out=dbgT[img, cc], in_=d["attnT"][:, cc])
                    nc.sync.dma_start(out=dbgC[img, cc], in_=c2xs[img][:, cc])

    nc.finalize()
    return nc


def _prep_inputs(inputs: dict) -> list[dict]:
    bf = ml_dtypes.bfloat16
    x = np.asarray(inputs["x"], dtype=np.float32)
    f32 = lambda k: np.asarray(inputs[k], dtype=np.float32)
    bn1_inv = f32("bn1_gamma") / np.sqrt(f32("bn1_var") + EPS)
    shift1 = f32("bn1_beta") - f32("bn1_mean") * bn1_inv + f32("conv1_b") * bn1_inv
    w1s = f32("conv1_w") * bn1_inv[:, None, None, None]
    bn2_inv = f32("bn2_gamma") / np.sqrt(f32("bn2_var") + EPS)
    shift2 = f32("bn2_beta") - f32("bn2_mean") * bn2_inv + f32("conv2_b") * bn2_inv
    w2s = f32("conv2_w") * bn2_inv[:, None, None, None]
    sg = 1.0 / (1.0 + np.exp(-float(np.asarray(inputs["gate"]))))
    ow = f32("out_w") * sg
    shiftF = shift2 + sg * f32("out_b") + sg * (f32("out_w") @ f32("v_b"))
    qws = f32("q_w") / np.sqrt(D)
    qbs = f32("q_b") / np.sqrt(D)

    def conv_pack(w):  # [O, I, 3, 3] -> [128, CC*9*CC*128]
        t = w.transpose(1, 2, 3, 0).reshape(CC, 128, 3, 3, CC, 128)
        return t.transpose(1, 0, 2, 3, 4, 5).reshape(128, W1_COLS)

    def pack_T(w):  # [O, C_in] -> [128, CC*C]
        return w.T.reshape(CC, 128, C).transpose(1, 0, 2).reshape(128, QKVO_COLS)

    wpack = np.concatenate(
        [conv_pack(w1s), conv_pack(w2s), pack_T(qws), pack_T(f32("k_w")),
         pack_T(f32("v_w")), pack_T(ow), np.eye(128, dtype=np.float32)],
        axis=1).astype(bf)
    assert wpack.shape == (128, PACK_COLS)

    vecs = np.concatenate(
        [np.stack([shift1.reshape(CC, 128), shiftF.reshape(CC, 128),
                   qbs.reshape(CC, 128)]).reshape(3 * CC, 128).T,
         np.eye(128, dtype=np.float32),
         np.full((128, 1), EXP_SHIFT, np.float32)], axis=1)
    assert vecs.shape == (128, VEC_COLS)
    shared = {"wpack": np.ascontiguousarray(wpack),
              "vecs": np.ascontiguousarray(vecs.astype(np.float32))}
    # pre-pad + pre-cast x to bf16: [B, CC, 128, HP, HP] with zero borders
    xp = np.zeros((B, CC, 128, HP, HP), dtype=bf)
    xp[:, :, :, 1:HP - 1, 1:HP - 1] = (
        x.reshape(B, CC, 128, H, W).astype(bf))
    xp = xp.reshape(B, CC, 128, HP * HP)
    in_maps = []
    for core in range(N_CORES):
        xs = xp[core * IMGS:(core + 1) * IMGS]
        in_maps.append({"x_sh": np.ascontiguousarray(xs), **shared})
    return in_maps


_NC_CACHE = {}


def _get_nc():
    if "nc" not in _NC_CACHE:
        _NC_CACHE["nc"] = build_nc()
    return _NC_CACHE["nc"]


def kernel(**inputs) -> np.ndarray:
    nc = _get_nc()
    in_maps = _prep_inputs(inputs)
    res = run_bass_kernel_spmd(nc, in_maps, core_ids=list(range(N_CORES)))
    outs = [res.results[i]["out_sh"].reshape(IMGS, C, H, W) for i in range(N_CORES)]
    return np.concatenate(outs, axis=0)



# revision 44
# speedup vs baseline: 1.0238x; 1.0173x over previous
"""Trainium2 Bass kernel for nn_AttentionResidualBlock (B=16, C=256, H=W=32, heads=8).

Sharding: data-parallel over batch across 8 NeuronCores (2 images/core),
weights replicated.

Per core (per image):
  - conv3x3 as 9 shifted bf16 matmuls over a zero-padded [C, 34, 34] layout;
    BN scale folded into weights on host, BN shift + ReLU fused on DVE.
    Conv work paces as TensorE filler inside the attention slots.
  - attention head-by-head: scoresT[m,n] = k^T q via fp8e4 DoubleRow
    matmuls (k carries a zeroed second k-tile; both operands fp8) at half
    the bf16 streaming cost; exp on ScalarE (PSUM -> SBUF bf16); attn@v
    computed TRANSPOSED: out[n, d] = sum_m pt[m, n] v[m, d] with
    lhsT = pt blocks and a ones-augmented v (33 cols) so the softmax
    denominator lands on the same partition as its outputs. Normalize is
    a per-partition reciprocal + tensor_scalar multiply. The 8 psum
    accumulation groups per head run ncb-outer/mc-inner (sequential per
    bank) because start=True lazily re-arms the whole 2KB zero region.
  - attnT is transposed back to [c, n] with DMA XBAR transposes
    (SBUF->SBUF, no PSUM), then a dense out-projection; gate/out-bias/
    v-bias folded on host.
Conv/attn@v/proj matmuls bf16, scores fp8-DR, fp32 PSUM accumulation.
"""

import os
import numpy as np
import ml_dtypes
from contextlib import ExitStack

KDBG = bool(int(os.environ.get("KDBG", "0")))

import concourse.bass as bass
import concourse.bacc as bacc
import concourse.mybir as mybir
import concourse.tile as tile
from concourse.bass_utils import run_bass_kernel_spmd

F32 = mybir.dt.float32
BF16 = mybir.dt.bfloat16
FP8 = mybir.dt.float8e4
AF = mybir.ActivationFunctionType
ALU = mybir.AluOpType
DR = mybir.MatmulPerfMode.DoubleRow
EXP_SHIFT = -4.0  # exp(s-4): keeps fp8e4 probabilities in range
# fp8e4 Schraudolph constants for exp(s-4): bits = s*11.5416 + 9.4896, with
# uint8 saturation mapping s < -0.82 to p=0 (verified on HW; negligible mass)
E4_MULT, E4_ADD = 11.5416, 56.0 - 4 * 11.5416 - 0.344
# exp slots alternate DVE (uint8 Schraudolph) / Act (real exp -> fp8):
# idx % EXP_MOD < EXP_DVE go to DVE
EXP_MOD, EXP_DVE = 5, 2


def _dr0(ap: bass.AP) -> bass.AP:
    """Insert a stride-0 dim after the partition dim (DoubleRow k-tile reuse)."""
    return bass.AP(tensor=ap.tensor, offset=ap.offset,
                   ap=[list(ap.ap[0])] + [[0, 2]] + [list(d) for d in ap.ap[1:]])

C = 256
HEADS = 8
D = 32
B, H, W = 16, 32, 32
N = H * W          # 1024
HP = H + 2         # 34
EPS = 1e-5
N_CORES = 8
IMGS = B // N_CORES  # 2 images per core
CC = C // 128      # 2 channel chunks
MC = N // 128      # 8 spatial m-chunks
NCH = 8            # n-chunks for attn output
DAUG = D + 1       # 33 (v cols + ones col)

# packed bf16 weight layout (columns per partition)
W1_COLS = CC * 9 * CC * 128          # 4608
QKVO_COLS = CC * C                   # 512
PACK_COLS = 2 * W1_COLS + 4 * QKVO_COLS + 128  # w1 w2 q k v ow ident = 11392
O_IDENT = PACK_COLS - 128            # bf16 identity for PE-mode transpose
VEC_COLS = 3 * CC + 128 + 1          # shift1, shiftF, qbias, identity(f32), exp-shift


def build_nc() -> bass.Bass:
    nc = bacc.Bacc()

    x_d = nc.declare_dram_parameter("x_sh", [IMGS, CC, 128, HP * HP], BF16,
                                    isOutput=False)
    wp_d = nc.declare_dram_parameter("wpack", [128, PACK_COLS], BF16, isOutput=False)
    vec_d = nc.declare_dram_parameter("vecs", [128, VEC_COLS], F32, isOutput=False)
    out_d = nc.declare_dram_parameter("out_sh", [IMGS, CC, 128, N], F32, isOutput=True)
    if KDBG:
        dbgA = nc.declare_dram_parameter("dbg_A", [IMGS, CC, 128, N], BF16,
                                         isOutput=True)
        dbgT = nc.declare_dram_parameter("dbg_attnT", [IMGS, CC, 128, N], BF16,
                                         isOutput=True)
        dbgC = nc.declare_dram_parameter("dbg_c2x", [IMGS, CC, 128, N], F32,
                                         isOutput=True)

    o_w1, o_w2 = 0, W1_COLS
    o_q = 2 * W1_COLS
    o_k, o_v = o_q + QKVO_COLS, o_q + 2 * QKVO_COLS
    o_ow = o_q + 3 * QKVO_COLS

    with ExitStack() as ctx:
        tc = ctx.enter_context(tile.TileContext(nc))
        wpool = ctx.enter_context(tc.tile_pool(name="weights", bufs=1))
        xpool = ctx.enter_context(tc.tile_pool(name="acts", bufs=2))
        ptpool = ctx.enter_context(tc.tile_pool(name="pt", bufs=10))
        epool = ctx.enter_context(tc.tile_pool(name="epi", bufs=5))
        ps_sc = ctx.enter_context(tc.tile_pool(name="ps_sc", bufs=4, space="PSUM"))
        ps_at = ctx.enter_context(tc.tile_pool(name="ps_at", bufs=2, space="PSUM"))
        ps_cv = ctx.enter_context(tc.tile_pool(name="ps_cv", bufs=2, space="PSUM"))

        # ---- weights / vectors ----
        wpack = wpool.tile([128, PACK_COLS], BF16, tag="wpack")
        vecs = wpool.tile([128, VEC_COLS], F32, tag="vecs")

        def conv_w(base, ic, tap, oc):  # [128, 128] lhsT slice
            off = base + ((ic * 9 + tap) * CC + oc) * 128
            return wpack[:, off:off + 128]

        shift1 = lambda oc: vecs[:, oc:oc + 1]
        shiftF = lambda oc: vecs[:, CC + oc:CC + oc + 1]
        qbias = lambda oc: vecs[:, 2 * CC + oc:2 * CC + oc + 1]
        ident = vecs[:, 3 * CC:3 * CC + 128]
        expshift = vecs[:, 3 * CC + 128:3 * CC + 129]

        # ---- filler queue (PE work units paced into attention slots) ----
        queue = []

        def push(cycles, fn, front=False):
            queue_cycles[0] += cycles
            if front:
                queue.insert(0, (cycles, fn))
            else:
                queue.append((cycles, fn))

        queue_cycles = [0]   # running total of cycles in queue
        slots_left = [128]   # attention slots remaining in the whole program

        RESERVE = 7000  # PE cycles held back to fill the post-last-exp tail

        def pop_fill():
            # spread remaining queue work evenly over remaining slots, but
            # never burst past the Act period (starves the exp pacer) nor
            # drip so slowly that conv debt piles up past the last exp; keep
            # RESERVE cycles back so the tail chain (norm/transpose/proj)
            # overlaps PE work instead of idling it
            avail = max(0, queue_cycles[0] - RESERVE)
            budget = min(1600, max(1200, avail // max(1, slots_left[0])))
            budget = min(budget, avail)
            slots_left[0] -= 1
            done = 0
            while queue and done < budget:
                cyc, fn = queue.pop(0)
                queue_cycles[0] -= cyc
                fn()
                done += cyc

        def drain_queue():
            while queue:
                _, fn = queue.pop(0)
                fn()

        # ---- per-image tiles ----
        # x arrives pre-padded + pre-cast to bf16 from the host: contiguous
        # DMA, no border memsets, no on-device casts
        xtiles = {}

        def xload_dma(img):
            xpadb = xpool.tile([128, CC, HP, HP], BF16, tag="xpadb",
                               name=f"xpadb{img}")
            xtiles[img] = (xpadb, xpadb)

            def chunk(cc):
                nc.sync.dma_start(
                    out=xpadb[:, cc].rearrange("p r c -> p (r c)"),
                    in_=x_d[img, cc])
            return chunk

        def xflat(t, cc):  # unpadded [p, 32, 32] view
            return t[:, cc, 1:HP - 1, 1:HP - 1]

        # ---- qkv ----
        qkv_tiles = {}

        def qkv_alloc(img):
            # vaug: [p, mc-pair, pair-slot, head, 48] fp8 (cols 0:32 = v, 32 =
            # ones; 48-stride keeps the DR pair step 16B-aligned)
            d = {
                "q": xpool.tile([128, CC, N], FP8, tag="q", name=f"q{img}"),
                "k": xpool.tile([128, CC, 2, N], FP8, tag="k", name=f"k{img}"),
                "xnb": xpool.tile([128, CC, N], BF16, tag="xnb", name=f"xnb{img}"),
                "vaug": xpool.tile([128, MC // 2, 2, HEADS, 48], FP8, tag="vaug",
                                   name=f"vaug{img}"),
                "attnT": xpool.tile([128, CC, NCH * 128], BF16, tag="attnT",
                                    name=f"attnT{img}"),
                "A": xpool.tile([128, CC, N], BF16, tag="A", name=f"A{img}"),
            }
            # zero second k-tiles for the DoubleRow zero-pad trick (on DVE:
            # Pool's sequencer must stay clear for the startup x casts)
            nc.vector.memset(d["k"][:, :, 1, :], 0.0)
            qkv_tiles[img] = d
            return d

        def qk_chunk(img, oc, which):
            d = qkv_tiles[img]
            xpadb = xtiles[img][1]
            wb = o_q if which == "q" else o_k
            for nh in range(2):
                ps = ps_sc.tile([128, 512], F32, tag="sc",
                                name=f"ps{which}{img}_{oc}_{nh}")
                for ic in range(CC):
                    nc.tensor.matmul(
                        ps,
                        lhsT=wpack[:, wb + ic * C + oc * 128:
                                   wb + ic * C + (oc + 1) * 128],
                        rhs=xflat(xpadb, ic)[:, nh * 16:(nh + 1) * 16, :],
                        start=(ic == 0), stop=(ic == CC - 1))
                if which == "q":
                    nc.scalar.activation(d["q"][:, oc, nh * 512:(nh + 1) * 512],
                                         ps, AF.Identity, bias=qbias(oc))
                else:
                    nc.vector.tensor_copy(d["k"][:, oc, 0, nh * 512:(nh + 1) * 512],
                                          ps)

        def v_chunk(img, half):
            d = qkv_tiles[img]
            xpadb = xtiles[img][1]
            if half == 0:
                nc.gpsimd.memset(d["vaug"][:, :, :, :, D], 1.0)
                for cc in range(CC):
                    nc.gpsimd.tensor_copy(
                        d["xnb"][:, cc].rearrange("p (r c) -> p r c", r=H),
                        xflat(xpadb, cc))
            for pair in range(2):
                ps = ps_sc.tile([128, 512], F32, tag="sc",
                                name=f"psv{img}_{half}_{pair}")
                for sl in range(2):
                    mc = half * 4 + pair * 2 + sl
                    for ic in range(CC):
                        nc.tensor.matmul(
                            ps[:, sl * C:(sl + 1) * C],
                            lhsT=d["xnb"][:, ic, mc * 128:(mc + 1) * 128],
                            rhs=wpack[:, o_v + ic * C: o_v + (ic + 1) * C],
                            start=(ic == 0), stop=(ic == CC - 1))
                for sl in range(2):
                    mc = half * 4 + pair * 2 + sl
                    nc.vector.tensor_copy(
                        d["vaug"][:, mc // 2, mc % 2, :, 0:D],
                        ps[:, sl * C:(sl + 1) * C].rearrange("p (h e) -> p h e",
                                                             h=HEADS))

        # ---- conv chains (filler units) ----
        def push_conv_units(img, cname, w_base, oc, nh):
            state = {}
            mmlist = [(ic, tap) for ic in range(CC) for tap in range(9)]

            def consume(ps):
                xpad, xpadb = xtiles[img]
                if cname == "c1":
                    # on Act: Relu(ps + shift1) — Act has slack since 1/3 of
                    # the exp stream moved to DVE
                    nc.scalar.activation(
                        xflat(c1pads[img], oc)[:, nh * 16:(nh + 1) * 16, :],
                        ps.rearrange("p (r c) -> p r c", r=16),
                        AF.Relu, bias=shift1(oc))
                else:
                    nc.vector.scalar_tensor_tensor(
                        out=c2xs[img][:, oc, nh * 512:(nh + 1) * 512]
                            .rearrange("p (r c) -> p r c", r=16),
                        in0=ps.rearrange("p (r c) -> p r c", r=16),
                        scalar=shiftF(oc),
                        in1=xflat(xpadb, oc)[:, nh * 16:(nh + 1) * 16, :],
                        op0=ALU.add, op1=ALU.add)

            def mk(i0, i1):
                def fn():
                    if "ps" not in state:
                        state["ps"] = ps_cv.tile([128, 512], F32, tag="cv",
                                                 name=f"{cname}{img}_{oc}_{nh}")
                    ps = state["ps"]
                    src = xtiles[img][1] if cname == "c1" else c1pads[img]
                    for idx in range(i0, i1):
                        ic, tap = mmlist[idx]
                        ky, kx = divmod(tap, 3)
                        nc.tensor.matmul(
                            ps,
                            lhsT=conv_w(w_base, ic, tap, oc),
                            rhs=src[:, ic, ky + nh * 16:ky + nh * 16 + 16, kx:kx + W],
                            start=(idx == 0), stop=(idx == 17))
                    if i1 == 18:
                        consume(ps)
                return fn

            for i0 in range(0, 18, 3):
                push(3 * 512, mk(i0, min(i0 + 3, 18)))

        c1pads, c2xs = {}, {}

        def conv_alloc(img):
            c1pad = xpool.tile([128, CC, HP, HP], BF16, tag="c1pad", name=f"c1p{img}")
            for cc in range(CC):
                nc.gpsimd.memset(c1pad[:, cc, 0, :], 0.0)
                nc.gpsimd.memset(c1pad[:, cc, HP - 1, :], 0.0)
                nc.gpsimd.memset(c1pad[:, cc, 1:HP - 1, 0], 0.0)
                nc.gpsimd.memset(c1pad[:, cc, 1:HP - 1, HP - 1], 0.0)
            c1pads[img] = c1pad
            c2xs[img] = xpool.tile([128, CC, N], F32, tag="c2x", name=f"c2x{img}")

        def push_conv_all(img):
            for oc in range(CC):
                for nh in range(2):
                    push_conv_units(img, "c1", o_w1, oc, nh)
            for oc in range(CC):
                for nh in range(2):
                    push_conv_units(img, "c2", o_w2, oc, nh)

        # ---- attention ----
        def emit_head_norm(img, h, at):
            # one broadcast multiply per head: out[p,g,c] = at[p,g,c]*rcp[p,g]
            d = qkv_tiles[img]
            rcp = xpool.tile([128, NCH], F32, tag="rcp", name=f"rcp{img}_{h}")
            nc.vector.reciprocal(
                rcp, at.rearrange("p (g e) -> p g e", e=DAUG)[:, :, D])
            rcp_bc = bass.AP(tensor=rcp.tensor, offset=rcp.offset,
                             ap=[list(rcp.ap[0])] + [[1, NCH], [0, D]])
            cch, hh = h // 4, h % 4
            nc.vector.scalar_tensor_tensor(
                out=d["attnT"][:, cch].rearrange("p (g c) -> p g c", c=128)
                    [:, :, hh * D:(hh + 1) * D],
                in0=at.rearrange("p (g e) -> p g e", e=DAUG)[:, :, 0:D],
                scalar=0.0,
                in1=rcp_bc,
                op0=ALU.add, op1=ALU.mult)

        def push_transp(img, cc):
            # batched DMA XBAR transpose: ONE instruction flips all 8 128x128
            # blocks of a cc-half (SBUF->SBUF, no PSUM)
            d = qkv_tiles[img]

            # emitted IMMEDIATELY at the trigger point (not queued): proj fns
            # are front-pushed and would otherwise emit before this transpose,
            # reading A before it is written (no dep in the Tile trace)
            if img == IMGS - 1 and cc == 1:
                # tail: PE-mode transpose + Act copy -- PE and Act are idle
                # here, and this skips the ~2.4us DMA/DGE latency chain
                ps = ps_sc.tile([128, N], BF16, tag="sc", name="trtail")
                for b in range(NCH):
                    nc.tensor.transpose(
                        ps[:, b * 128:(b + 1) * 128],
                        d["attnT"][:, cc, b * 128:(b + 1) * 128],
                        wpack[:, O_IDENT:O_IDENT + 128])
                nc.vector.tensor_copy(d["A"][:, cc], ps)
            else:
                nc.sync.dma_start_transpose(
                    out=d["A"][:, cc].rearrange("p (a b) -> p a b", a=NCH),
                    in_=d["attnT"][:, cc])

        def push_proj(img):
            d = qkv_tiles[img]
            for oc in range(CC):
                for nh in range(2):
                    def fn(oc=oc, nh=nh):
                        # img1's proj runs at the tail when the score banks are
                        # free: 4-deep rotation lets all 4 groups pipeline
                        pool = ps_sc if img == IMGS - 1 else ps_cv
                        pj = pool.tile([128, 512], F32, tag="sc" if img == IMGS - 1 else "cv",
                                       name=f"pj{img}_{oc}_{nh}")
                        for cc in range(CC):
                            nc.tensor.matmul(
                                pj,
                                lhsT=wpack[:, o_ow + cc * C + oc * 128:
                                           o_ow + cc * C + oc * 128 + 128],
                                rhs=d["A"][:, cc, nh * 512:(nh + 1) * 512],
                                start=(cc == 0), stop=(cc == CC - 1))
                        cmb = epool.tile([128, 512], F32, tag="cmb",
                                         name=f"cmb{img}_{oc}_{nh}")
                        nc.vector.scalar_tensor_tensor(
                            out=cmb, in0=pj, scalar=0.0,
                            in1=c2xs[img][:, oc, nh * 512:(nh + 1) * 512],
                            op0=ALU.add, op1=ALU.add)
                        osb = epool.tile([128, 512], F32, tag="osb",
                                         name=f"osb{img}_{oc}_{nh}")
                        nc.gpsimd.tensor_scalar(osb, cmb, 0.0, None, ALU.max)
                        nc.sync.dma_start(
                            out=out_d[img, oc, :, nh * 512:(nh + 1) * 512], in_=osb)
                    # img0's proj flows as immediate filler; img1's drains at
                    # the tail after the conv reserve covers transpose latency
                    push(1024, fn, front=(img == 0))

        # attention sub-blocks: one (head, ncb) group per slot, lagged one
        # full head so all 8 pt tiles of the head exist. Groups are strictly
        # SEQUENTIAL within the psum bank (ncb outer, mc inner): TRN2's
        # start=True lazily re-arms the whole 2KB zero region, so interleaved
        # per-group starts would wipe other groups' partial accumulations.
        attn_subs = []
        at_tiles = {}

        def make_attn_subs(img, h, pts):
            d = qkv_tiles[img]

            def mk(ncb):
                def fn():
                    if ncb == 0:
                        at_tiles[(img, h)] = ps_at.tile(
                            [128, NCH * DAUG], F32, tag="at", name=f"at{img}_{h}")
                    at = at_tiles[(img, h)]
                    for pr in range(MC // 2):
                        # DoubleRow over a REAL mc-pair: pt pair tile is the
                        # lhsT k-pair, vaug pair-slots are the rhs
                        nc.tensor.matmul(
                            at[:, ncb * DAUG:(ncb + 1) * DAUG],
                            lhsT=pts[pr][:, :, ncb * 128:(ncb + 1) * 128],
                            rhs=d["vaug"][:, pr, :, h, 0:DAUG],
                            start=(pr == 0), stop=(pr == MC // 2 - 1),
                            perf_mode=DR, skip_group_check=True)
                    if ncb == NCH - 1:
                        emit_head_norm(img, h, at)
                        del at_tiles[(img, h)]
                        if h == 3:
                            push_transp(img, 0)
                        if h == 7:
                            push_transp(img, 1)
                            push_proj(img)
                return fn

            for ncb in range(NCH):
                attn_subs.append(mk(ncb))

        def head_loop(img):
            d = qkv_tiles[img]
            q_sb, k_sb = d["q"], d["k"]
            for h in range(HEADS):
                hp, cch = 32 * (h % 4), h // 4
                pts = []
                for mc in range(MC):
                    if mc % 2 == 0:
                        pts.append(ptpool.tile([128, 2, N], FP8, tag="pt",
                                               name=f"pt{img}_{h}_{mc // 2}"))
                    for nh in range(2):
                        # half-tile score psum (one bank) -> 4-deep rotation so
                        # the exp latency stays off the matmul critical path
                        sc = ps_sc.tile([128, 512], F32, tag="sc",
                                        name=f"sc{img}_{h}_{mc}_{nh}")
                        nc.tensor.matmul(
                            sc,
                            lhsT=k_sb[hp:hp + 32, cch, :, mc * 128:(mc + 1) * 128],
                            rhs=_dr0(q_sb[hp:hp + 32, cch,
                                          nh * 512:(nh + 1) * 512]),
                            start=True, stop=True, perf_mode=DR,
                            tile_position=(hp, 0))
                        slot = pts[-1][:, mc % 2, nh * 512:(nh + 1) * 512]
                        idx = ((img * HEADS + h) * MC + mc) * 2 + nh
                        if idx % EXP_MOD < EXP_DVE:
                            # fp8e4 Schraudolph on DVE: linear-bits in uint8;
                            # saturation maps s-4 < -10.4 to p=0 (tiny mass)
                            nc.vector.tensor_scalar(
                                slot.bitcast(mybir.dt.uint8), sc,
                                E4_MULT, E4_ADD, ALU.mult, ALU.add)
                        else:
                            nc.scalar.activation(slot, sc, AF.Exp, bias=expshift)
                    if attn_subs:
                        attn_subs.pop(0)()
                    pop_fill()
                make_attn_subs(img, h, pts)

        # ================= emission =================
        # DMA order matters: transfers serialize on the DMA engines, so the
        # small qkv weights + vecs go first (first consumers), then image 0,
        # then the two conv-weight halves, then image 1.
        chunk0 = xload_dma(0)
        nc.sync.dma_start(out=wpack[:, o_q:], in_=wp_d[:, o_q:])
        chunk0(0)
        chunk0(1)
        nc.sync.dma_start(out=vecs, in_=vec_d[:])
        nc.sync.dma_start(out=wpack[:, :W1_COLS], in_=wp_d[:, :W1_COLS])
        nc.sync.dma_start(out=wpack[:, W1_COLS:o_q], in_=wp_d[:, W1_COLS:o_q])
        chunk1 = xload_dma(1)
        chunk1(0)
        chunk1(1)

        qkv_alloc(0)
        qk_chunk(0, 0, "q")
        qk_chunk(0, 0, "k")
        push(0, lambda: conv_alloc(0))
        push(2048, lambda: v_chunk(0, 0), front=True)
        push(2048, lambda: v_chunk(0, 1))
        push(2048, lambda: qk_chunk(0, 1, "q"))
        push(2048, lambda: qk_chunk(0, 1, "k"))
        push_conv_all(0)
        # image 1 prep as filler inside image 0's slots
        push(0, lambda: (qkv_alloc(1), conv_alloc(1)) and None)
        push(2048, lambda: qk_chunk(1, 0, "q"))
        push(2048, lambda: qk_chunk(1, 0, "k"))
        push(2048, lambda: v_chunk(1, 0))
        push(2048, lambda: v_chunk(1, 1))
        push(2048, lambda: qk_chunk(1, 1, "q"))
        push(2048, lambda: qk_chunk(1, 1, "k"))

        head_loop(0)
        push_conv_all(1)
        head_loop(1)
        while attn_subs:
            attn_subs.pop(0)()
        drain_queue()
        if KDBG:
            for img in range(IMGS):
                d = qkv_tiles[img]
                for cc in range(CC):
                    nc.sync.dma_start(out=dbgA[img, cc], in_=d["A"][:, cc])
                    nc.sync.dma_start(out=dbgT[img, cc], in_=d["attnT"][:, cc])
                    nc.sync.dma_start(out=dbgC[img, cc], in_=c2xs[img][:, cc])

    nc.finalize()
    return nc


def _prep_inputs(inputs: dict) -> list[dict]:
    bf = ml_dtypes.bfloat16
    x = np.asarray(inputs["x"], dtype=np.float32)
    f32 = lambda k: np.asarray(inputs[k], dtype=np.float32)
    bn1_inv = f32("bn1_gamma") / np.sqrt(f32("bn1_var") + EPS)
    shift1 = f32("bn1_beta") - f32("bn1_mean") * bn1_inv + f32("conv1_b") * bn1_inv
    w1s = f32("conv1_w") * bn1_inv[:, None, None, None]
    bn2_inv = f32("bn2_gamma") / np.sqrt(f32("bn2_var") + EPS)
    shift2 = f32("bn2_beta") - f32("bn2_mean") * bn2_inv + f32("conv2_b") * bn2_inv
    w2s = f32("conv2_w") * bn2_inv[:, None, None, None]
    sg = 1.0 / (1.0 + np.exp(-float(np.asarray(inputs["gate"]))))
    ow = f32("out_w") * sg
    shiftF = shift2 + sg * f32("out_b") + sg * (f32("out_w") @ f32("v_b"))
    qws = f32("q_w") / np.sqrt(D)
    qbs = f32("q_b") / np.sqrt(D)

    def conv_pack(w):  # [O, I, 3, 3] -> [128, CC*9*CC*128]
        t = w.transpose(1, 2, 3, 0).reshape(CC, 128, 3, 3, CC, 128)
        return t.transpose(1, 0, 2, 3, 4, 5).reshape(128, W1_COLS)

    def pack_T(w):  # [O, C_in] -> [128, CC*C]
        return w.T.reshape(CC, 128, C).transpose(1, 0, 2).reshape(128, QKVO_COLS)

    wpack = np.concatenate(
        [conv_pack(w1s), conv_pack(w2s), pack_T(qws), pack_T(f32("k_w")),
         pack_T(f32("v_w")), pack_T(ow), np.eye(128, dtype=np.float32)],
        axis=1).astype(bf)
    assert wpack.shape == (128, PACK_COLS)

    vecs = np.concatenate(
        [np.stack([shift1.reshape(CC, 128), shiftF.reshape(CC, 128),
                   qbs.reshape(CC, 128)]).reshape(3 * CC, 128).T,
         np.eye(128, dtype=np.float32),
         np.full((128, 1), EXP_SHIFT, np.float32)], axis=1)
    assert vecs.shape == (128, VEC_COLS)
    shared = {"wpack": np.ascontiguousarray(wpack),
              "vecs": np.ascontiguousarray(vecs.astype(np.float32))}
    # pre-pad + pre-cast x to bf16: [B, CC, 128, HP, HP] with zero borders
    xp = np.zeros((B, CC, 128, HP, HP), dtype=bf)
    xp[:, :, :, 1:HP - 1, 1:HP - 1] = (
        x.reshape(B, CC, 128, H, W).astype(bf))
    xp = xp.reshape(B, CC, 128, HP * HP)
    in_maps = []
    for core in range(N_CORES):
        xs = xp[core * IMGS:(core + 1) * IMGS]
        in_maps.append({"x_sh": np.ascontiguousarray(xs), **shared})
    return in_maps


_NC_CACHE = {}


def _get_nc():
    if "nc" not in _NC_CACHE:
        _NC_CACHE["nc"] = build_nc()
    return _NC_CACHE["nc"]


def kernel(**inputs) -> np.ndarray:
    nc = _get_nc()
    in_maps = _prep_inputs(inputs)
    res = run_bass_kernel_spmd(nc, in_maps, core_ids=list(range(N_CORES)))
    outs = [res.results[i]["out_sh"].reshape(IMGS, C, H, W) for i in range(N_CORES)]
    return np.concatenate(outs, axis=0)



# revision 45
# speedup vs baseline: 1.0253x; 1.0015x over previous
"""Trainium2 Bass kernel for nn_AttentionResidualBlock (B=16, C=256, H=W=32, heads=8).

Sharding: data-parallel over batch across 8 NeuronCores (2 images/core),
weights replicated.

Per core (per image):
  - conv3x3 as 9 shifted bf16 matmuls over a zero-padded [C, 34, 34] layout;
    BN scale folded into weights on host, BN shift + ReLU fused on DVE.
    Conv work paces as TensorE filler inside the attention slots.
  - attention head-by-head: scoresT[m,n] = k^T q via fp8e4 DoubleRow
    matmuls (k carries a zeroed second k-tile; both operands fp8) at half
    the bf16 streaming cost; exp on ScalarE (PSUM -> SBUF bf16); attn@v
    computed TRANSPOSED: out[n, d] = sum_m pt[m, n] v[m, d] with
    lhsT = pt blocks and a ones-augmented v (33 cols) so the softmax
    denominator lands on the same partition as its outputs. Normalize is
    a per-partition reciprocal + tensor_scalar multiply. The 8 psum
    accumulation groups per head run ncb-outer/mc-inner (sequential per
    bank) because start=True lazily re-arms the whole 2KB zero region.
  - attnT is transposed back to [c, n] with DMA XBAR transposes
    (SBUF->SBUF, no PSUM), then a dense out-projection; gate/out-bias/
    v-bias folded on host.
Conv/attn@v/proj matmuls bf16, scores fp8-DR, fp32 PSUM accumulation.
"""

import os
import numpy as np
import ml_dtypes
from contextlib import ExitStack

KDBG = bool(int(os.environ.get("KDBG", "0")))

import concourse.bass as bass
import concourse.bacc as bacc
import concourse.mybir as mybir
import concourse.tile as tile
from concourse.bass_utils import run_bass_kernel_spmd

F32 = mybir.dt.float32
BF16 = mybir.dt.bfloat16
FP8 = mybir.dt.float8e4
AF = mybir.ActivationFunctionType
ALU = mybir.AluOpType
DR = mybir.MatmulPerfMode.DoubleRow
EXP_SHIFT = -4.0  # exp(s-4): keeps fp8e4 probabilities in range
# fp8e4 Schraudolph constants for exp(s-4): bits = s*11.5416 + 9.4896, with
# uint8 saturation mapping s < -0.82 to p=0 (verified on HW; negligible mass)
E4_MULT, E4_ADD = 11.5416, 56.0 - 4 * 11.5416 - 0.344
# exp slots alternate DVE (uint8 Schraudolph) / Act (real exp -> fp8):
# idx % EXP_MOD < EXP_DVE go to DVE
EXP_MOD, EXP_DVE = 5, 2


def _dr0(ap: bass.AP) -> bass.AP:
    """Insert a stride-0 dim after the partition dim (DoubleRow k-tile reuse)."""
    return bass.AP(tensor=ap.tensor, offset=ap.offset,
                   ap=[list(ap.ap[0])] + [[0, 2]] + [list(d) for d in ap.ap[1:]])

C = 256
HEADS = 8
D = 32
B, H, W = 16, 32, 32
N = H * W          # 1024
HP = H + 2         # 34
EPS = 1e-5
N_CORES = 8
IMGS = B // N_CORES  # 2 images per core
CC = C // 128      # 2 channel chunks
MC = N // 128      # 8 spatial m-chunks
NCH = 8            # n-chunks for attn output
DAUG = D + 1       # 33 (v cols + ones col)

# packed bf16 weight layout (columns per partition)
W1_COLS = CC * 9 * CC * 128          # 4608
QKVO_COLS = CC * C                   # 512
PACK_COLS = 2 * W1_COLS + 4 * QKVO_COLS + 128  # w1 w2 q k v ow ident = 11392
O_IDENT = PACK_COLS - 128            # bf16 identity for PE-mode transpose
VEC_COLS = 3 * CC + 128 + 1          # shift1, shiftF, qbias, identity(f32), exp-shift


def build_nc() -> bass.Bass:
    nc = bacc.Bacc()

    x_d = nc.declare_dram_parameter("x_sh", [IMGS, CC, 128, HP * HP], BF16,
                                    isOutput=False)
    wp_d = nc.declare_dram_parameter("wpack", [128, PACK_COLS], BF16, isOutput=False)
    vec_d = nc.declare_dram_parameter("vecs", [128, VEC_COLS], F32, isOutput=False)
    out_d = nc.declare_dram_parameter("out_sh", [IMGS, CC, 128, N], F32, isOutput=True)
    if KDBG:
        dbgA = nc.declare_dram_parameter("dbg_A", [IMGS, CC, 128, N], BF16,
                                         isOutput=True)
        dbgT = nc.declare_dram_parameter("dbg_attnT", [IMGS, CC, 128, N], BF16,
                                         isOutput=True)
        dbgC = nc.declare_dram_parameter("dbg_c2x", [IMGS, CC, 128, N], F32,
                                         isOutput=True)

    o_w1, o_w2 = 0, W1_COLS
    o_q = 2 * W1_COLS
    o_k, o_v = o_q + QKVO_COLS, o_q + 2 * QKVO_COLS
    o_ow = o_q + 3 * QKVO_COLS

    with ExitStack() as ctx:
        tc = ctx.enter_context(tile.TileContext(nc))
        wpool = ctx.enter_context(tc.tile_pool(name="weights", bufs=1))
        xpool = ctx.enter_context(tc.tile_pool(name="acts", bufs=2))
        ptpool = ctx.enter_context(tc.tile_pool(name="pt", bufs=10))
        epool = ctx.enter_context(tc.tile_pool(name="epi", bufs=5))
        ps_sc = ctx.enter_context(tc.tile_pool(name="ps_sc", bufs=4, space="PSUM"))
        ps_at = ctx.enter_context(tc.tile_pool(name="ps_at", bufs=2, space="PSUM"))
        ps_cv = ctx.enter_context(tc.tile_pool(name="ps_cv", bufs=2, space="PSUM"))

        # ---- weights / vectors ----
        wpack = wpool.tile([128, PACK_COLS], BF16, tag="wpack")
        vecs = wpool.tile([128, VEC_COLS], F32, tag="vecs")

        def conv_w(base, ic, tap, oc):  # [128, 128] lhsT slice
            off = base + ((ic * 9 + tap) * CC + oc) * 128
            return wpack[:, off:off + 128]

        shift1 = lambda oc: vecs[:, oc:oc + 1]
        shiftF = lambda oc: vecs[:, CC + oc:CC + oc + 1]
        qbias = lambda oc: vecs[:, 2 * CC + oc:2 * CC + oc + 1]
        ident = vecs[:, 3 * CC:3 * CC + 128]
        expshift = vecs[:, 3 * CC + 128:3 * CC + 129]

        # ---- filler queue (PE work units paced into attention slots) ----
        queue = []

        def push(cycles, fn, front=False):
            queue_cycles[0] += cycles
            if front:
                queue.insert(0, (cycles, fn))
            else:
                queue.append((cycles, fn))

        queue_cycles = [0]   # running total of cycles in queue
        slots_left = [128]   # attention slots remaining in the whole program

        RESERVE = 10000  # PE cycles held back to fill the post-last-exp tail

        def pop_fill():
            # spread remaining queue work evenly over remaining slots, but
            # never burst past the Act period (starves the exp pacer) nor
            # drip so slowly that conv debt piles up past the last exp; keep
            # RESERVE cycles back so the tail chain (norm/transpose/proj)
            # overlaps PE work instead of idling it
            avail = max(0, queue_cycles[0] - RESERVE)
            budget = min(1600, max(1200, avail // max(1, slots_left[0])))
            budget = min(budget, avail)
            slots_left[0] -= 1
            done = 0
            while queue and done < budget:
                cyc, fn = queue.pop(0)
                queue_cycles[0] -= cyc
                fn()
                done += cyc

        def drain_queue():
            while queue:
                _, fn = queue.pop(0)
                fn()

        # ---- per-image tiles ----
        # x arrives pre-padded + pre-cast to bf16 from the host: contiguous
        # DMA, no border memsets, no on-device casts
        xtiles = {}

        def xload_dma(img):
            xpadb = xpool.tile([128, CC, HP, HP], BF16, tag="xpadb",
                               name=f"xpadb{img}")
            xtiles[img] = (xpadb, xpadb)

            def chunk(cc):
                nc.sync.dma_start(
                    out=xpadb[:, cc].rearrange("p r c -> p (r c)"),
                    in_=x_d[img, cc])
            return chunk

        def xflat(t, cc):  # unpadded [p, 32, 32] view
            return t[:, cc, 1:HP - 1, 1:HP - 1]

        # ---- qkv ----
        qkv_tiles = {}

        def qkv_alloc(img):
            # vaug: [p, mc-pair, pair-slot, head, 48] fp8 (cols 0:32 = v, 32 =
            # ones; 48-stride keeps the DR pair step 16B-aligned)
            d = {
                "q": xpool.tile([128, CC, N], FP8, tag="q", name=f"q{img}"),
                "k": xpool.tile([128, CC, 2, N], FP8, tag="k", name=f"k{img}"),
                "xnb": xpool.tile([128, CC, N], BF16, tag="xnb", name=f"xnb{img}"),
                "vaug": xpool.tile([128, MC // 2, 2, HEADS, 48], FP8, tag="vaug",
                                   name=f"vaug{img}"),
                "attnT": xpool.tile([128, CC, NCH * 128], BF16, tag="attnT",
                                    name=f"attnT{img}"),
                "A": xpool.tile([128, CC, N], BF16, tag="A", name=f"A{img}"),
            }
            # zero second k-tiles for the DoubleRow zero-pad trick (on DVE:
            # Pool's sequencer must stay clear for the startup x casts)
            nc.vector.memset(d["k"][:, :, 1, :], 0.0)
            qkv_tiles[img] = d
            return d

        def qk_chunk(img, oc, which):
            d = qkv_tiles[img]
            xpadb = xtiles[img][1]
            wb = o_q if which == "q" else o_k
            for nh in range(2):
                ps = ps_sc.tile([128, 512], F32, tag="sc",
                                name=f"ps{which}{img}_{oc}_{nh}")
                for ic in range(CC):
                    nc.tensor.matmul(
                        ps,
                        lhsT=wpack[:, wb + ic * C + oc * 128:
                                   wb + ic * C + (oc + 1) * 128],
                        rhs=xflat(xpadb, ic)[:, nh * 16:(nh + 1) * 16, :],
                        start=(ic == 0), stop=(ic == CC - 1))
                if which == "q":
                    nc.scalar.activation(d["q"][:, oc, nh * 512:(nh + 1) * 512],
                                         ps, AF.Identity, bias=qbias(oc))
                else:
                    nc.vector.tensor_copy(d["k"][:, oc, 0, nh * 512:(nh + 1) * 512],
                                          ps)

        def v_chunk(img, half):
            d = qkv_tiles[img]
            xpadb = xtiles[img][1]
            if half == 0:
                nc.gpsimd.memset(d["vaug"][:, :, :, :, D], 1.0)
                for cc in range(CC):
                    nc.gpsimd.tensor_copy(
                        d["xnb"][:, cc].rearrange("p (r c) -> p r c", r=H),
                        xflat(xpadb, cc))
            for pair in range(2):
                ps = ps_sc.tile([128, 512], F32, tag="sc",
                                name=f"psv{img}_{half}_{pair}")
                for sl in range(2):
                    mc = half * 4 + pair * 2 + sl
                    for ic in range(CC):
                        nc.tensor.matmul(
                            ps[:, sl * C:(sl + 1) * C],
                            lhsT=d["xnb"][:, ic, mc * 128:(mc + 1) * 128],
                            rhs=wpack[:, o_v + ic * C: o_v + (ic + 1) * C],
                            start=(ic == 0), stop=(ic == CC - 1))
                for sl in range(2):
                    mc = half * 4 + pair * 2 + sl
                    nc.vector.tensor_copy(
                        d["vaug"][:, mc // 2, mc % 2, :, 0:D],
                        ps[:, sl * C:(sl + 1) * C].rearrange("p (h e) -> p h e",
                                                             h=HEADS))

        # ---- conv chains (filler units) ----
        def push_conv_units(img, cname, w_base, oc, nh):
            state = {}
            mmlist = [(ic, tap) for ic in range(CC) for tap in range(9)]

            def consume(ps):
                xpad, xpadb = xtiles[img]
                if cname == "c1":
                    # on Act: Relu(ps + shift1) — Act has slack since 1/3 of
                    # the exp stream moved to DVE
                    nc.scalar.activation(
                        xflat(c1pads[img], oc)[:, nh * 16:(nh + 1) * 16, :],
                        ps.rearrange("p (r c) -> p r c", r=16),
                        AF.Relu, bias=shift1(oc))
                else:
                    nc.vector.scalar_tensor_tensor(
                        out=c2xs[img][:, oc, nh * 512:(nh + 1) * 512]
                            .rearrange("p (r c) -> p r c", r=16),
                        in0=ps.rearrange("p (r c) -> p r c", r=16),
                        scalar=shiftF(oc),
                        in1=xflat(xpadb, oc)[:, nh * 16:(nh + 1) * 16, :],
                        op0=ALU.add, op1=ALU.add)

            def mk(i0, i1):
                def fn():
                    if "ps" not in state:
                        state["ps"] = ps_cv.tile([128, 512], F32, tag="cv",
                                                 name=f"{cname}{img}_{oc}_{nh}")
                    ps = state["ps"]
                    src = xtiles[img][1] if cname == "c1" else c1pads[img]
                    for idx in range(i0, i1):
                        ic, tap = mmlist[idx]
                        ky, kx = divmod(tap, 3)
                        nc.tensor.matmul(
                            ps,
                            lhsT=conv_w(w_base, ic, tap, oc),
                            rhs=src[:, ic, ky + nh * 16:ky + nh * 16 + 16, kx:kx + W],
                            start=(idx == 0), stop=(idx == 17))
                    if i1 == 18:
                        consume(ps)
                return fn

            for i0 in range(0, 18, 3):
                push(3 * 512, mk(i0, min(i0 + 3, 18)))

        c1pads, c2xs = {}, {}

        def conv_alloc(img):
            c1pad = xpool.tile([128, CC, HP, HP], BF16, tag="c1pad", name=f"c1p{img}")
            for cc in range(CC):
                nc.gpsimd.memset(c1pad[:, cc, 0, :], 0.0)
                nc.gpsimd.memset(c1pad[:, cc, HP - 1, :], 0.0)
                nc.gpsimd.memset(c1pad[:, cc, 1:HP - 1, 0], 0.0)
                nc.gpsimd.memset(c1pad[:, cc, 1:HP - 1, HP - 1], 0.0)
            c1pads[img] = c1pad
            c2xs[img] = xpool.tile([128, CC, N], F32, tag="c2x", name=f"c2x{img}")

        def push_conv_all(img):
            for oc in range(CC):
                for nh in range(2):
                    push_conv_units(img, "c1", o_w1, oc, nh)
            for oc in range(CC):
                for nh in range(2):
                    push_conv_units(img, "c2", o_w2, oc, nh)

        # ---- attention ----
        def emit_head_norm(img, h, at):
            # one broadcast multiply per head: out[p,g,c] = at[p,g,c]*rcp[p,g]
            d = qkv_tiles[img]
            rcp = xpool.tile([128, NCH], F32, tag="rcp", name=f"rcp{img}_{h}")
            nc.vector.reciprocal(
                rcp, at.rearrange("p (g e) -> p g e", e=DAUG)[:, :, D])
            rcp_bc = bass.AP(tensor=rcp.tensor, offset=rcp.offset,
                             ap=[list(rcp.ap[0])] + [[1, NCH], [0, D]])
            cch, hh = h // 4, h % 4
            nc.vector.scalar_tensor_tensor(
                out=d["attnT"][:, cch].rearrange("p (g c) -> p g c", c=128)
                    [:, :, hh * D:(hh + 1) * D],
                in0=at.rearrange("p (g e) -> p g e", e=DAUG)[:, :, 0:D],
                scalar=0.0,
                in1=rcp_bc,
                op0=ALU.add, op1=ALU.mult)

        def push_transp(img, cc):
            # batched DMA XBAR transpose: ONE instruction flips all 8 128x128
            # blocks of a cc-half (SBUF->SBUF, no PSUM)
            d = qkv_tiles[img]

            # emitted IMMEDIATELY at the trigger point (not queued): proj fns
            # are front-pushed and would otherwise emit before this transpose,
            # reading A before it is written (no dep in the Tile trace)
            if img == IMGS - 1 and cc == 1:
                # tail: PE-mode transpose + Act copy -- PE and Act are idle
                # here, and this skips the ~2.4us DMA/DGE latency chain
                ps = ps_sc.tile([128, N], BF16, tag="sc", name="trtail")
                for b in range(NCH):
                    nc.tensor.transpose(
                        ps[:, b * 128:(b + 1) * 128],
                        d["attnT"][:, cc, b * 128:(b + 1) * 128],
                        wpack[:, O_IDENT:O_IDENT + 128])
                nc.vector.tensor_copy(d["A"][:, cc], ps)
            else:
                nc.sync.dma_start_transpose(
                    out=d["A"][:, cc].rearrange("p (a b) -> p a b", a=NCH),
                    in_=d["attnT"][:, cc])

        def push_proj(img):
            d = qkv_tiles[img]
            for oc in range(CC):
                for nh in range(2):
                    def fn(oc=oc, nh=nh):
                        # img1's proj runs at the tail when the score banks are
                        # free: 4-deep rotation lets all 4 groups pipeline
                        pool = ps_sc if img == IMGS - 1 else ps_cv
                        pj = pool.tile([128, 512], F32, tag="sc" if img == IMGS - 1 else "cv",
                                       name=f"pj{img}_{oc}_{nh}")
                        for cc in range(CC):
                            nc.tensor.matmul(
                                pj,
                                lhsT=wpack[:, o_ow + cc * C + oc * 128:
                                           o_ow + cc * C + oc * 128 + 128],
                                rhs=d["A"][:, cc, nh * 512:(nh + 1) * 512],
                                start=(cc == 0), stop=(cc == CC - 1))
                        cmb = epool.tile([128, 512], F32, tag="cmb",
                                         name=f"cmb{img}_{oc}_{nh}")
                        nc.vector.scalar_tensor_tensor(
                            out=cmb, in0=pj, scalar=0.0,
                            in1=c2xs[img][:, oc, nh * 512:(nh + 1) * 512],
                            op0=ALU.add, op1=ALU.add)
                        osb = epool.tile([128, 512], F32, tag="osb",
                                         name=f"osb{img}_{oc}_{nh}")
                        nc.gpsimd.tensor_scalar(osb, cmb, 0.0, None, ALU.max)
                        nc.sync.dma_start(
                            out=out_d[img, oc, :, nh * 512:(nh + 1) * 512], in_=osb)
                    # img0's proj flows as immediate filler; img1's drains at
                    # the tail after the conv reserve covers transpose latency
                    push(1024, fn, front=(img == 0))

        # attention sub-blocks: one (head, ncb) group per slot, lagged one
        # full head so all 8 pt tiles of the head exist. Groups are strictly
        # SEQUENTIAL within the psum bank (ncb outer, mc inner): TRN2's
        # start=True lazily re-arms the whole 2KB zero region, so interleaved
        # per-group starts would wipe other groups' partial accumulations.
        attn_subs = []
        at_tiles = {}

        def make_attn_subs(img, h, pts):
            d = qkv_tiles[img]

            def mk(ncb):
                def fn():
                    if ncb == 0:
                        at_tiles[(img, h)] = ps_at.tile(
                            [128, NCH * DAUG], F32, tag="at", name=f"at{img}_{h}")
                    at = at_tiles[(img, h)]
                    for pr in range(MC // 2):
                        # DoubleRow over a REAL mc-pair: pt pair tile is the
                        # lhsT k-pair, vaug pair-slots are the rhs
                        nc.tensor.matmul(
                            at[:, ncb * DAUG:(ncb + 1) * DAUG],
                            lhsT=pts[pr][:, :, ncb * 128:(ncb + 1) * 128],
                            rhs=d["vaug"][:, pr, :, h, 0:DAUG],
                            start=(pr == 0), stop=(pr == MC // 2 - 1),
                            perf_mode=DR, skip_group_check=True)
                    if ncb == NCH - 1:
                        emit_head_norm(img, h, at)
                        del at_tiles[(img, h)]
                        if h == 3:
                            push_transp(img, 0)
                        if h == 7:
                            push_transp(img, 1)
                            push_proj(img)
                return fn

            for ncb in range(NCH):
                attn_subs.append(mk(ncb))

        def head_loop(img):
            d = qkv_tiles[img]
            q_sb, k_sb = d["q"], d["k"]
            for h in range(HEADS):
                hp, cch = 32 * (h % 4), h // 4
                pts = []
                for mc in range(MC):
                    if mc % 2 == 0:
                        pts.append(ptpool.tile([128, 2, N], FP8, tag="pt",
                                               name=f"pt{img}_{h}_{mc // 2}"))
                    for nh in range(2):
                        # half-tile score psum (one bank) -> 4-deep rotation so
                        # the exp latency stays off the matmul critical path
                        sc = ps_sc.tile([128, 512], F32, tag="sc",
                                        name=f"sc{img}_{h}_{mc}_{nh}")
                        nc.tensor.matmul(
                            sc,
                            lhsT=k_sb[hp:hp + 32, cch, :, mc * 128:(mc + 1) * 128],
                            rhs=_dr0(q_sb[hp:hp + 32, cch,
                                          nh * 512:(nh + 1) * 512]),
                            start=True, stop=True, perf_mode=DR,
                            tile_position=(hp, 0))
                        slot = pts[-1][:, mc % 2, nh * 512:(nh + 1) * 512]
                        idx = ((img * HEADS + h) * MC + mc) * 2 + nh
                        if idx % EXP_MOD < EXP_DVE:
                            # fp8e4 Schraudolph on DVE: linear-bits in uint8;
                            # saturation maps s-4 < -10.4 to p=0 (tiny mass)
                            nc.vector.tensor_scalar(
                                slot.bitcast(mybir.dt.uint8), sc,
                                E4_MULT, E4_ADD, ALU.mult, ALU.add)
                        else:
                            nc.scalar.activation(slot, sc, AF.Exp, bias=expshift)
                    if attn_subs:
                        attn_subs.pop(0)()
                    pop_fill()
                make_attn_subs(img, h, pts)

        # ================= emission =================
        # DMA order matters: transfers serialize on the DMA engines, so the
        # small qkv weights + vecs go first (first consumers), then image 0,
        # then the two conv-weight halves, then image 1.
        chunk0 = xload_dma(0)
        nc.sync.dma_start(out=wpack[:, o_q:], in_=wp_d[:, o_q:])
        chunk0(0)
        chunk0(1)
        nc.sync.dma_start(out=vecs, in_=vec_d[:])
        nc.sync.dma_start(out=wpack[:, :W1_COLS], in_=wp_d[:, :W1_COLS])
        nc.sync.dma_start(out=wpack[:, W1_COLS:o_q], in_=wp_d[:, W1_COLS:o_q])
        chunk1 = xload_dma(1)
        chunk1(0)
        chunk1(1)

        qkv_alloc(0)
        qk_chunk(0, 0, "q")
        qk_chunk(0, 0, "k")
        push(0, lambda: conv_alloc(0))
        push(2048, lambda: v_chunk(0, 0), front=True)
        push(2048, lambda: v_chunk(0, 1))
        push(2048, lambda: qk_chunk(0, 1, "q"))
        push(2048, lambda: qk_chunk(0, 1, "k"))
        push_conv_all(0)
        # image 1 prep as filler inside image 0's slots
        push(0, lambda: (qkv_alloc(1), conv_alloc(1)) and None)
        push(2048, lambda: qk_chunk(1, 0, "q"))
        push(2048, lambda: qk_chunk(1, 0, "k"))
        push(2048, lambda: v_chunk(1, 0))
        push(2048, lambda: v_chunk(1, 1))
        push(2048, lambda: qk_chunk(1, 1, "q"))
        push(2048, lambda: qk_chunk(1, 1, "k"))

        head_loop(0)
        push_conv_all(1)
        head_loop(1)
        while attn_subs:
            attn_subs.pop(0)()
        drain_queue()
        if KDBG:
            for img in range(IMGS):
                d = qkv_tiles[img]
                for cc in range(CC):
                    nc.sync.dma_start(out=dbgA[img, cc], in_=d["A"][:, cc])
                    nc.sync.dma_start(out=dbgT[img, cc], in_=d["attnT"][:, cc])
                    nc.sync.dma_start(out=dbgC[img, cc], in_=c2xs[img][:, cc])

    nc.finalize()
    return nc


def _prep_inputs(inputs: dict) -> list[dict]:
    bf = ml_dtypes.bfloat16
    x = np.asarray(inputs["x"], dtype=np.float32)
    f32 = lambda k: np.asarray(inputs[k], dtype=np.float32)
    bn1_inv = f32("bn1_gamma") / np.sqrt(f32("bn1_var") + EPS)
    shift1 = f32("bn1_beta") - f32("bn1_mean") * bn1_inv + f32("conv1_b") * bn1_inv
    w1s = f32("conv1_w") * bn1_inv[:, None, None, None]
    bn2_inv = f32("bn2_gamma") / np.sqrt(f32("bn2_var") + EPS)
    shift2 = f32("bn2_beta") - f32("bn2_mean") * bn2_inv + f32("conv2_b") * bn2_inv
    w2s = f32("conv2_w") * bn2_inv[:, None, None, None]
    sg = 1.0 / (1.0 + np.exp(-float(np.asarray(inputs["gate"]))))
    ow = f32("out_w") * sg
    shiftF = shift2 + sg * f32("out_b") + sg * (f32("out_w") @ f32("v_b"))
    qws = f32("q_w") / np.sqrt(D)
    qbs = f32("q_b") / np.sqrt(D)

    def conv_pack(w):  # [O, I, 3, 3] -> [128, CC*9*CC*128]
        t = w.transpose(1, 2, 3, 0).reshape(CC, 128, 3, 3, CC, 128)
        return t.transpose(1, 0, 2, 3, 4, 5).reshape(128, W1_COLS)

    def pack_T(w):  # [O, C_in] -> [128, CC*C]
        return w.T.reshape(CC, 128, C).transpose(1, 0, 2).reshape(128, QKVO_COLS)

    wpack = np.concatenate(
        [conv_pack(w1s), conv_pack(w2s), pack_T(qws), pack_T(f32("k_w")),
         pack_T(f32("v_w")), pack_T(ow), np.eye(128, dtype=np.float32)],
        axis=1).astype(bf)
    assert wpack.shape == (128, PACK_COLS)

    vecs = np.concatenate(
        [np.stack([shift1.reshape(CC, 128), shiftF.reshape(CC, 128),
                   qbs.reshape(CC, 128)]).reshape(3 * CC, 128).T,
         np.eye(128, dtype=np.float32),
         np.full((128, 1), EXP_SHIFT, np.float32)], axis=1)
    assert vecs.shape == (128, VEC_COLS)
    shared = {"wpack": np.ascontiguousarray(wpack),
              "vecs": np.ascontiguousarray(vecs.astype(np.float32))}
    # pre-pad + pre-cast x to bf16: [B, CC, 128, HP, HP] with zero borders
    xp = np.zeros((B, CC, 128, HP, HP), dtype=bf)
    xp[:, :, :, 1:HP - 1, 1:HP - 1] = (
        x.reshape(B, CC, 128, H, W).astype(bf))
    xp = xp.reshape(B, CC, 128, HP * HP)
    in_maps = []
    for core in range(N_CORES):
        xs = xp[core * IMGS:(core + 1) * IMGS]
        in_maps.append({"x_sh": np.ascontiguousarray(xs), **shared})
    return in_maps


_NC_CACHE = {}


def _get_nc():
    if "nc" not in _NC_CACHE:
        _NC_CACHE["nc"] = build_nc()
    return _NC_CACHE["nc"]


def kernel(**inputs) -> np.ndarray:
    nc = _get_nc()
    in_maps = _prep_inputs(inputs)
    res = run_bass_kernel_spmd(nc, in_maps, core_ids=list(range(N_CORES)))
    outs = [res.results[i]["out_sh"].reshape(IMGS, C, H, W) for i in range(N_CORES)]
    return np.concatenate(outs, axis=0)

